# revision 1
# baseline (speedup 1.0000x reference)
"""Two-layer GCN encoder (GCNConv x2 -> mu/logvar heads) on 8 TRN2 NeuronCores.

v2: the module is linear (no activation between convs), so it collapses to
  [mu | lv] = A^2 @ X @ Wc,   Wc = W1 @ W2 @ [W_mu | W_lv]  (256 x 64)
with A = D^-1/2 (Adj + I) D^-1/2. Folding the normalization row-wise:
  z0 = dinv * (X @ Wc)          (sharded: each core computes its 6250 rows)
  z1 = invdeg * (Ahat @ z0)     (SpMM over edges incl self-loops)
  Y  = dinv * (Ahat @ z1)       (second SpMM; Y[:, :32] = mu, Y[:, 32:] = lv)
Nonzero biases are corrected host-side via the rank-1 identity
  Y += s * (b1 W2 Wh) + (b2 Wh + bh),  s = A @ 1.

Device structure:
  - z0/z1 tables are [N, 128] bf16 (cols 0:64 = data, rest zero) so gather
    rows stay 256B; tables are exchanged with AllGather after each stage.
  - Edges partitioned by destination core, sorted by (src-half, dst);
    messages fetched with dma_gather (int16 idx -> <=32767-row halves) on
    4 SWDGE queues; scatter-add into PSUM via one-hot matmul with the
    one-hot as lhsT: acc[dst, f] += sel^T @ msg, so PSUM is node-major and
    feeds the next table (or the output) directly after a per-row scale.
"""

import os

import ml_dtypes
import numpy as np

import concourse.bacc as bacc
import concourse.bass as bass
import concourse.mybir as mybir
import concourse.tile as tile
from concourse import library_config
from concourse.bass_utils import run_bass_kernel_spmd

# ---- problem constants (hardcoded per harness contract) ----
N = 50000
IN_D, HID1, HID2, OUT_D = 256, 128, 64, 32
NC_CORES = 8
NSH = N // NC_CORES  # 6250 dst nodes per core
NBLK = (NSH + 127) // 128  # 49 dst blocks per core
HALF = 24960  # block-aligned split; both halves < 32767 (int16 gather idx)
NBLK_ALL = -(-N // 128)  # 391 node blocks total
CHUNK_BLOCKS = 3  # dst blocks per gather chunk
FC = 64  # collapsed feature count

BF16 = ml_dtypes.bfloat16

_tile_patched = False


def _patch_tile_drain():
    """walrus in this env rejects >~2 sem waits on one instruction; Tile's
    kernel-tail drain aggregates one wait per live semaphore. Move the excess
    onto dedicated single-wait SP nops that precede the drain."""
    global _tile_patched
    if _tile_patched:
        return
    _tile_patched = True
    _orig = tile.TileContext._drain_and_barrier

    def _patched(self, tick_clock, wait_clock):
        nc = self.nc
        nops = [nc.sync.nop(nofuse=True, hint=f"dw_{i}").ins for i in range(64)]
        _orig(self, tick_clock, wait_clock)
        ni = 0
        for inst in nc.cur_bb.bb.instructions:
            if "Drain" not in type(inst).__name__:
                continue
            ow = inst.sync_info.on_wait if inst.sync_info else []
            if len(ow) > 1:
                waits = list(ow)
                for w in waits[:-1]:
                    nops[ni].sync_info = mybir.SyncInfo(on_wait=[w], on_update=[])
                    ni += 1
                inst.sync_info.on_wait[:] = waits[-1:]

    tile.TileContext._drain_and_barrier = _patched


def _prep(x, edge_index, W1, b1, W2, b2, W_mu, b_mu, W_lv, b_lv):
    """Host-side graph partitioning + input staging. Returns (in_maps, plan)."""
    src = np.asarray(edge_index[0], dtype=np.int64)
    dst = np.asarray(edge_index[1], dtype=np.int64)
    loop = np.arange(N, dtype=np.int64)
    src_a = np.concatenate([src, loop])
    dst_a = np.concatenate([dst, loop])

    deg = np.bincount(dst_a, minlength=N).astype(np.float64)
    dinv = deg**-0.5
    invdeg = 1.0 / deg

    # sort edges by (src-half, dst): each (dst-block, half) group contiguous
    half = (src_a >= HALF).astype(np.int64)
    key = half * N + dst_a
    order = np.argsort(key, kind="stable")
    s_sorted = src_a[order]
    d_sorted = dst_a[order]
    bnd = np.searchsorted(key[order], np.arange(2 * N + 1))

    # per-(core, block, half) counts -> core-independent tile counts
    T = [[0, 0] for _ in range(NBLK)]
    counts = np.zeros((NC_CORES, NBLK, 2), dtype=np.int64)
    for c in range(NC_CORES):
        for b in range(NBLK):
            lo = c * NSH + b * 128
            hi = min(c * NSH + (b + 1) * 128, (c + 1) * NSH)
            for h in range(2):
                counts[c, b, h] = bnd[h * N + hi] - bnd[h * N + lo]
    for b in range(NBLK):
        for h in range(2):
            T[b][h] = max(1, int(-(-counts[:, b, h].max() // 128)))

    TH = [sum(T[b][h] for b in range(NBLK)) for h in range(2)]
    toff = [[0] * NBLK, [0] * NBLK]
    for h in range(2):
        acc = 0
        for b in range(NBLK):
            toff[h][b] = acc
            acc += T[b][h]

    # per-core padded idx / dstloc streams
    core_data = []
    for c in range(NC_CORES):
        idx_streams = []
        dl_streams = []
        for h in range(2):
            idx = np.zeros(TH[h] * 128, dtype=np.int16)
            dl = np.full(TH[h] * 128, -1.0, dtype=np.float32)
            for b in range(NBLK):
                lo = c * NSH + b * 128
                hi = min(c * NSH + (b + 1) * 128, (c + 1) * NSH)
                e0, e1 = bnd[h * N + lo], bnd[h * N + hi]
                cnt = e1 - e0
                off = toff[h][b] * 128
                idx[off : off + cnt] = (s_sorted[e0:e1] - h * HALF).astype(np.int16)
                dl[off : off + cnt] = (d_sorted[e0:e1] - lo).astype(np.float32)
            packed = np.tile(np.ascontiguousarray(idx.reshape(-1, 16).T), (8, 1))
            idx_streams.append(packed)
            dl_streams.append(np.ascontiguousarray(dl.reshape(-1, 128).T).astype(BF16))
        core_data.append((idx_streams, dl_streams))

    # collapsed weights
    W1_ = np.asarray(W1, np.float64)
    W2_ = np.asarray(W2, np.float64)
    Wh = np.concatenate(
        [np.asarray(W_mu, np.float64), np.asarray(W_lv, np.float64)], axis=1
    )  # [64, 64]
    Wc = W1_ @ W2_ @ Wh  # [256, 64]
    wca = Wc[:128].astype(BF16)
    wcb = Wc[128:].astype(BF16)

    # host-side bias correction (zero for this module)
    r1 = (np.asarray(b1, np.float64) @ W2_) @ Wh  # [64]
    r0 = np.asarray(b2, np.float64) @ Wh + np.concatenate(
        [np.asarray(b_mu, np.float64), np.asarray(b_lv, np.float64)]
    )
    if np.any(r1) or np.any(r0):
        s_vec = dinv * np.bincount(dst_a, weights=dinv[src_a], minlength=N)
        bias_corr = (s_vec[:, None] * r1[None, :] + r0[None, :]).astype(np.float32)
    else:
        bias_corr = None

    iota_rep = np.tile(np.arange(128, dtype=np.float32), (128, 8)).astype(BF16)

    xf = np.asarray(x, np.float32)
    in_maps = []
    for c in range(NC_CORES):
        (idxA, idxB), (dlA, dlB) = core_data[c]
        own = slice(c * NSH, (c + 1) * NSH)
        xsh = np.zeros((IN_D, NBLK * 128), np.float32)
        xsh[:, :NSH] = xf[own].T
        tmp_iv = np.zeros(NBLK * 128, np.float64)
        tmp_dv = np.zeros(NBLK * 128, np.float64)
        tmp_iv[:NSH] = invdeg[own]
        tmp_dv[:NSH] = dinv[own]
        in_maps.append(
            {
                "xsh": xsh.astype(BF16),
                "iota": iota_rep,
                "idxA": idxA,
                "idxB": idxB,
                "dlA": dlA,
                "dlB": dlB,
                "wca": wca,
                "wcb": wcb,
                "ivcol": np.ascontiguousarray(
                    tmp_iv.reshape(NBLK, 128).T
                ).astype(np.float32),
                "dvcol": np.ascontiguousarray(
                    tmp_dv.reshape(NBLK, 128).T
                ).astype(np.float32),
            }
        )

    plan = {"T": T, "TH": TH, "toff": toff, "bias_corr": bias_corr}
    return in_maps, plan


def _build(plan):
    _patch_tile_drain()
    T, TH, toff = plan["T"], plan["TH"], plan["toff"]

    nc = bacc.Bacc("TRN2", num_swdge_queues=4)
    f32, bf16, i16 = mybir.dt.float32, mybir.dt.bfloat16, mybir.dt.int16
    COPY = mybir.ActivationFunctionType.Copy

    xsh_e = nc.dram_tensor("xsh", [IN_D, NBLK * 128], bf16, kind="ExternalInput")
    iota_e = nc.dram_tensor("iota", [128, 1024], bf16, kind="ExternalInput")
    idxA_e = nc.dram_tensor("idxA", [128, TH[0] * 8], i16, kind="ExternalInput")
    idxB_e = nc.dram_tensor("idxB", [128, TH[1] * 8], i16, kind="ExternalInput")
    dlA_e = nc.dram_tensor("dlA", [128, TH[0]], bf16, kind="ExternalInput")
    dlB_e = nc.dram_tensor("dlB", [128, TH[1]], bf16, kind="ExternalInput")
    wca_e = nc.dram_tensor("wca", [128, FC], bf16, kind="ExternalInput")
    wcb_e = nc.dram_tensor("wcb", [128, FC], bf16, kind="ExternalInput")
    ivcol_e = nc.dram_tensor("ivcol", [128, NBLK], f32, kind="ExternalInput")
    dvcol_e = nc.dram_tensor("dvcol", [128, NBLK], f32, kind="ExternalInput")

    out_e = nc.dram_tensor("out", [NSH, FC], f32, kind="ExternalOutput")

    z0l_d = nc.dram_tensor("z0l_d", [NSH, 128], bf16)
    z0f_d = nc.dram_tensor("z0f_d", [N, 128], bf16, addr_space="Shared")
    z1l_d = nc.dram_tensor("z1l_d", [NSH, 128], bf16)
    z1f_d = nc.dram_tensor("z1f_d", [N, 128], bf16, addr_space="Shared")

    core_ids = list(range(NC_CORES))

    chunks = []
    b0 = 0
    while b0 < NBLK:
        chunks.append((b0, min(b0 + CHUNK_BLOCKS, NBLK)))
        b0 = min(b0 + CHUNK_BLOCKS, NBLK)

    with tile.TileContext(nc) as tc:
        with (
            tc.tile_pool(name="const", bufs=1) as pc,
            tc.tile_pool(name="xa", bufs=3) as px,
            tc.tile_pool(name="zb", bufs=4) as pz,
            tc.tile_pool(name="g", bufs=6) as pg,
            tc.tile_pool(name="sel", bufs=8) as psel,
            tc.tile_pool(name="psA", bufs=2, space="PSUM") as ppA,
            tc.tile_pool(name="psacc", bufs=4, space="PSUM") as ppa,
        ):
            nc.gpsimd.load_library(library_config.mlp)

            # ---- resident constants
            iota_t = pc.tile([128, 1024], bf16)
            nc.sync.dma_start(out=iota_t[:], in_=iota_e[:])
            idxA_t = pc.tile([128, TH[0] * 8], i16)
            nc.sync.dma_start(out=idxA_t[:], in_=idxA_e[:])
            idxB_t = pc.tile([128, TH[1] * 8], i16)
            nc.sync.dma_start(out=idxB_t[:], in_=idxB_e[:])
            dlA_t = pc.tile([128, TH[0]], bf16)
            nc.sync.dma_start(out=dlA_t[:], in_=dlA_e[:])
            dlB_t = pc.tile([128, TH[1]], bf16)
            nc.sync.dma_start(out=dlB_t[:], in_=dlB_e[:])
            wca_t = pc.tile([128, FC], bf16)
            nc.sync.dma_start(out=wca_t[:], in_=wca_e[:])
            wcb_t = pc.tile([128, FC], bf16)
            nc.sync.dma_start(out=wcb_t[:], in_=wcb_e[:])
            ivcol_t = pc.tile([128, NBLK], f32)
            nc.sync.dma_start(out=ivcol_t[:], in_=ivcol_e[:])
            dvcol_t = pc.tile([128, NBLK], f32)
            nc.sync.dma_start(out=dvcol_t[:], in_=dvcol_e[:])

            # ---- phase A: z0 shard = dinv * (x_shard @ Wc)
            with nc.named_scope("phaseA"):
                done = 0
                while done < NBLK:
                    nb_cnt = min(8, NBLK - done)
                    c0 = done * 128
                    cols = nb_cnt * 128
                    xa = px.tile([128, 1024], bf16, tag="xa")
                    xb = px.tile([128, 1024], bf16, tag="xb")
                    nc.sync.dma_start(
                        out=xa[:, :cols], in_=xsh_e[0:128, c0 : c0 + cols]
                    )
                    nc.scalar.dma_start(
                        out=xb[:, :cols], in_=xsh_e[128:256, c0 : c0 + cols]
                    )
                    for j in range(nb_cnt):
                        gb = done + j
                        rows = min(128, NSH - gb * 128)
                        zp = ppA.tile([128, FC], f32, space="PSUM", tag="zp")
                        nc.tensor.matmul(
                            out=zp[:],
                            lhsT=xa[:, j * 128 : (j + 1) * 128],
                            rhs=wca_t[:],
                            start=True,
                            stop=False,
                        )
                        nc.tensor.matmul(
                            out=zp[:],
                            lhsT=xb[:, j * 128 : (j + 1) * 128],
                            rhs=wcb_t[:],
                            start=False,
                            stop=True,
                        )
                        z0s = pz.tile([128, 128], bf16, tag="z0s")
                        nc.vector.memset(z0s[:, FC:128], 0.0)
                        nc.scalar.activation(
                            z0s[:, 0:FC], zp[:], COPY,
                            scale=dvcol_t[:, gb : gb + 1],
                        )
                        nc.sync.dma_start(
                            out=z0l_d[gb * 128 : gb * 128 + rows], in_=z0s[:rows]
                        )
                    done += nb_cnt

            with nc.named_scope("ag0"):
                nc.gpsimd.collective_compute(
                    "AllGather",
                    mybir.AluOpType.bypass,
                    ins=[z0l_d[:]],
                    outs=[z0f_d[:]],
                    replica_groups=[core_ids],
                )

            # ---- shared SpMM: out[dst_block] = scale * sum_e msg[e]
            def spmm(srcA, srcB, scale_t, store, qoff=0):
                qn = qoff
                for (cb0, cb1) in chunks:
                    ctA = sum(T[b][0] for b in range(cb0, cb1))
                    ctB = sum(T[b][1] for b in range(cb0, cb1))
                    offA, offB = toff[0][cb0], toff[1][cb0]
                    gA = pg.tile([128, ctA, 128], bf16, tag="gA")
                    gB = pg.tile([128, ctB, 128], bf16, tag="gB")
                    nc.gpsimd.dma_gather(
                        gA[:], srcA, idxA_t[:, offA * 8 : (offA + ctA) * 8],
                        ctA * 128, ctA * 128, 128,
                        single_packet=False, queue_num=qn % 4,
                    )
                    nc.gpsimd.dma_gather(
                        gB[:], srcB, idxB_t[:, offB * 8 : (offB + ctB) * 8],
                        ctB * 128, ctB * 128, 128,
                        single_packet=False, queue_num=(qn + 1) % 4,
                    )
                    qn += 2
                    for b in range(cb0, cb1):
                        acc = ppa.tile([128, FC], f32, space="PSUM", tag="acc")
                        n_mm = T[b][0] + T[b][1]
                        mi = 0
                        for h, g, dl_t, off0 in (
                            (0, gA, dlA_t, offA),
                            (1, gB, dlB_t, offB),
                        ):
                            tloc0 = toff[h][b] - off0
                            nt = T[b][h]
                            done = 0
                            while done < nt:
                                k = min(8, nt - done)
                                sel = psel.tile([128, k, 128], bf16, tag="sel")
                                nc.vector.tensor_tensor(
                                    out=sel[:],
                                    in0=dl_t[
                                        :, toff[h][b] + done : toff[h][b] + done + k
                                    ].to_broadcast([128, k, 128]),
                                    in1=iota_t[:, : k * 128],
                                    op=mybir.AluOpType.is_equal,
                                )
                                for q in range(k):
                                    nc.tensor.matmul(
                                        out=acc[:],
                                        lhsT=sel[:, q, :],
                                        rhs=g[:, tloc0 + done + q, 0:FC],
                                        start=(mi == 0),
                                        stop=(mi == n_mm - 1),
                                    )
                                    mi += 1
                                done += k
                        store(b, acc, scale_t)

            # ---- SpMM1 -> z1 table shard; AllGather
            def store_z1(b, acc, scale_t):
                rows = min(128, NSH - b * 128)
                z1s = pz.tile([128, 128], bf16, tag="z1s")
                nc.vector.memset(z1s[:, FC:128], 0.0)
                nc.scalar.activation(
                    z1s[:, 0:FC], acc[:], COPY, scale=scale_t[:, b : b + 1]
                )
                nc.sync.dma_start(
                    out=z1l_d[b * 128 : b * 128 + rows], in_=z1s[:rows]
                )

            with nc.named_scope("spmm1"):
                spmm(z0f_d[0:HALF], z0f_d[HALF:N], ivcol_t, store_z1, qoff=0)

            with nc.named_scope("ag1"):
                nc.gpsimd.collective_compute(
                    "AllGather",
                    mybir.AluOpType.bypass,
                    ins=[z1l_d[:]],
                    outs=[z1f_d[:]],
                    replica_groups=[core_ids],
                )

            # ---- SpMM2 -> final output
            def store_out(b, acc, scale_t):
                rows = min(128, NSH - b * 128)
                os_ = pz.tile([128, FC], f32, tag="outs")
                nc.scalar.activation(
                    os_[:], acc[:], COPY, scale=scale_t[:, b : b + 1]
                )
                nc.sync.dma_start(
                    out=out_e[b * 128 : b * 128 + rows], in_=os_[:rows]
                )

            with nc.named_scope("spmm2"):
                spmm(z1f_d[0:HALF], z1f_d[HALF:N], dvcol_t, store_out, qoff=2)

    nc.compile()
    return nc


_CACHE = {}


def kernel(**inputs):
    in_maps, plan = _prep(**inputs)
    key = tuple(tuple(t) for t in plan["T"])
    if key not in _CACHE:
        _CACHE[key] = _build(plan)
    nc = _CACHE[key]
    res = run_bass_kernel_spmd(nc, in_maps, list(range(NC_CORES)))
    out = np.concatenate(
        [res.results[c]["out"] for c in range(NC_CORES)], axis=0
    )
    if plan["bias_corr"] is not None:
        out = out + plan["bias_corr"]
    mu = np.ascontiguousarray(out[:, :OUT_D])
    lv = np.ascontiguousarray(out[:, OUT_D:])
    return (mu, lv)



# revision 6
# speedup vs baseline: 1.1705x; 1.1705x over previous
"""Two-layer GCN encoder (GCNConv x2 -> mu/logvar heads) on 8 TRN2 NeuronCores.

v2: the module is linear (no activation between convs), so it collapses to
  [mu | lv] = A^2 @ X @ Wc,   Wc = W1 @ W2 @ [W_mu | W_lv]  (256 x 64)
with A = D^-1/2 (Adj + I) D^-1/2. Folding the normalization row-wise:
  z0 = dinv * (X @ Wc)          (sharded: each core computes its 6250 rows)
  z1 = invdeg * (Ahat @ z0)     (SpMM over edges incl self-loops)
  Y  = dinv * (Ahat @ z1)       (second SpMM; Y[:, :32] = mu, Y[:, 32:] = lv)
Nonzero biases are corrected host-side via the rank-1 identity
  Y += s * (b1 W2 Wh) + (b2 Wh + bh),  s = A @ 1.

Device structure:
  - z0/z1 tables are [N, 128] bf16 (cols 0:64 = data, rest zero) so gather
    rows stay 256B; tables are exchanged with AllGather after each stage.
  - Edges partitioned by destination core, sorted by (src-half, dst);
    messages fetched with dma_gather (int16 idx -> <=32767-row halves) on
    4 SWDGE queues; scatter-add into PSUM via one-hot matmul with the
    one-hot as lhsT: acc[dst, f] += sel^T @ msg, so PSUM is node-major and
    feeds the next table (or the output) directly after a per-row scale.
"""

import os

import ml_dtypes
import numpy as np

import concourse.bacc as bacc
import concourse.bass as bass
import concourse.mybir as mybir
import concourse.tile as tile
from concourse import library_config
from concourse.bass_utils import run_bass_kernel_spmd

# ---- problem constants (hardcoded per harness contract) ----
N = 50000
IN_D, HID1, HID2, OUT_D = 256, 128, 64, 32
NC_CORES = 8
NSH = N // NC_CORES  # 6250 dst nodes per core
NBLK = (NSH + 127) // 128  # 49 dst blocks per core
HALF = 24960  # block-aligned split; both halves < 32767 (int16 gather idx)
NBLK_ALL = -(-N // 128)  # 391 node blocks total
CHUNK_BLOCKS = 3  # dst blocks per gather chunk
FC = 64  # collapsed feature count

BF16 = ml_dtypes.bfloat16

_tile_patched = False


def _patch_tile_drain():
    """walrus in this env rejects >~2 sem waits on one instruction; Tile's
    kernel-tail drain aggregates one wait per live semaphore. Move the excess
    onto dedicated single-wait SP nops that precede the drain."""
    global _tile_patched
    if _tile_patched:
        return
    _tile_patched = True
    _orig = tile.TileContext._drain_and_barrier

    def _patched(self, tick_clock, wait_clock):
        nc = self.nc
        nops = [nc.sync.nop(nofuse=True, hint=f"dw_{i}").ins for i in range(64)]
        _orig(self, tick_clock, wait_clock)
        ni = 0
        for inst in nc.cur_bb.bb.instructions:
            if "Drain" not in type(inst).__name__:
                continue
            ow = inst.sync_info.on_wait if inst.sync_info else []
            if len(ow) > 1:
                waits = list(ow)
                for w in waits[:-1]:
                    nops[ni].sync_info = mybir.SyncInfo(on_wait=[w], on_update=[])
                    ni += 1
                inst.sync_info.on_wait[:] = waits[-1:]

    tile.TileContext._drain_and_barrier = _patched


def _prep(x, edge_index, W1, b1, W2, b2, W_mu, b_mu, W_lv, b_lv):
    """Host-side graph partitioning + input staging. Returns (in_maps, plan)."""
    src = np.asarray(edge_index[0], dtype=np.int64)
    dst = np.asarray(edge_index[1], dtype=np.int64)
    loop = np.arange(N, dtype=np.int64)
    src_a = np.concatenate([src, loop])
    dst_a = np.concatenate([dst, loop])

    deg = np.bincount(dst_a, minlength=N).astype(np.float64)
    dinv = deg**-0.5
    invdeg = 1.0 / deg

    # sort edges by (src-half, dst): each (dst-block, half) group contiguous
    half = (src_a >= HALF).astype(np.int64)
    key = half * N + dst_a
    order = np.argsort(key, kind="stable")
    s_sorted = src_a[order]
    d_sorted = dst_a[order]
    bnd = np.searchsorted(key[order], np.arange(2 * N + 1))

    # per-(core, block, half) counts -> core-independent tile counts
    T = [[0, 0] for _ in range(NBLK)]
    counts = np.zeros((NC_CORES, NBLK, 2), dtype=np.int64)
    for c in range(NC_CORES):
        for b in range(NBLK):
            lo = c * NSH + b * 128
            hi = min(c * NSH + (b + 1) * 128, (c + 1) * NSH)
            for h in range(2):
                counts[c, b, h] = bnd[h * N + hi] - bnd[h * N + lo]
    for b in range(NBLK):
        for h in range(2):
            T[b][h] = max(1, int(-(-counts[:, b, h].max() // 128)))

    TH = [sum(T[b][h] for b in range(NBLK)) for h in range(2)]
    toff = [[0] * NBLK, [0] * NBLK]
    for h in range(2):
        acc = 0
        for b in range(NBLK):
            toff[h][b] = acc
            acc += T[b][h]

    # per-core padded idx / dstloc streams
    core_data = []
    for c in range(NC_CORES):
        idx_streams = []
        dl_streams = []
        for h in range(2):
            idx = np.zeros(TH[h] * 128, dtype=np.int16)
            dl = np.full(TH[h] * 128, -1.0, dtype=np.float32)
            for b in range(NBLK):
                lo = c * NSH + b * 128
                hi = min(c * NSH + (b + 1) * 128, (c + 1) * NSH)
                e0, e1 = bnd[h * N + lo], bnd[h * N + hi]
                cnt = e1 - e0
                off = toff[h][b] * 128
                idx[off : off + cnt] = (s_sorted[e0:e1] - h * HALF).astype(np.int16)
                dl[off : off + cnt] = (d_sorted[e0:e1] - lo).astype(np.float32)
            packed = np.tile(np.ascontiguousarray(idx.reshape(-1, 16).T), (8, 1))
            idx_streams.append(packed)
            dl_streams.append(np.ascontiguousarray(dl.reshape(-1, 128).T).astype(BF16))
        core_data.append((idx_streams, dl_streams))

    # collapsed weights
    W1_ = np.asarray(W1, np.float64)
    W2_ = np.asarray(W2, np.float64)
    Wh = np.concatenate(
        [np.asarray(W_mu, np.float64), np.asarray(W_lv, np.float64)], axis=1
    )  # [64, 64]
    Wc = W1_ @ W2_ @ Wh  # [256, 64]
    wca = Wc[:128].astype(BF16)
    wcb = Wc[128:].astype(BF16)

    # host-side bias correction (zero for this module)
    r1 = (np.asarray(b1, np.float64) @ W2_) @ Wh  # [64]
    r0 = np.asarray(b2, np.float64) @ Wh + np.concatenate(
        [np.asarray(b_mu, np.float64), np.asarray(b_lv, np.float64)]
    )
    if np.any(r1) or np.any(r0):
        s_vec = dinv * np.bincount(dst_a, weights=dinv[src_a], minlength=N)
        bias_corr = (s_vec[:, None] * r1[None, :] + r0[None, :]).astype(np.float32)
    else:
        bias_corr = None

    iota_rep = np.tile(np.arange(128, dtype=np.float32), (128, 8)).astype(BF16)

    xf = np.asarray(x, np.float32)
    in_maps = []
    for c in range(NC_CORES):
        (idxA, idxB), (dlA, dlB) = core_data[c]
        own = slice(c * NSH, (c + 1) * NSH)
        xsh = np.zeros((IN_D, NBLK * 128), np.float32)
        xsh[:, :NSH] = xf[own].T
        tmp_iv = np.zeros(NBLK * 128, np.float64)
        tmp_dv = np.zeros(NBLK * 128, np.float64)
        tmp_iv[:NSH] = invdeg[own]
        tmp_dv[:NSH] = dinv[own]
        in_maps.append(
            {
                "xsh": xsh.astype(BF16),
                "iota": iota_rep,
                "idxA": idxA,
                "idxB": idxB,
                "dlA": dlA,
                "dlB": dlB,
                "wca": wca,
                "wcb": wcb,
                "ivcol": np.ascontiguousarray(
                    tmp_iv.reshape(NBLK, 128).T
                ).astype(np.float32),
                "dvcol": np.ascontiguousarray(
                    tmp_dv.reshape(NBLK, 128).T
                ).astype(np.float32),
            }
        )

    plan = {"T": T, "TH": TH, "toff": toff, "bias_corr": bias_corr}
    return in_maps, plan


def _build(plan):
    _patch_tile_drain()
    T, TH, toff = plan["T"], plan["TH"], plan["toff"]

    nc = bacc.Bacc("TRN2", num_swdge_queues=4)
    f32, bf16, i16 = mybir.dt.float32, mybir.dt.bfloat16, mybir.dt.int16
    COPY = mybir.ActivationFunctionType.Copy

    xsh_e = nc.dram_tensor("xsh", [IN_D, NBLK * 128], bf16, kind="ExternalInput")
    iota_e = nc.dram_tensor("iota", [128, 1024], bf16, kind="ExternalInput")
    idxA_e = nc.dram_tensor("idxA", [128, TH[0] * 8], i16, kind="ExternalInput")
    idxB_e = nc.dram_tensor("idxB", [128, TH[1] * 8], i16, kind="ExternalInput")
    dlA_e = nc.dram_tensor("dlA", [128, TH[0]], bf16, kind="ExternalInput")
    dlB_e = nc.dram_tensor("dlB", [128, TH[1]], bf16, kind="ExternalInput")
    wca_e = nc.dram_tensor("wca", [128, FC], bf16, kind="ExternalInput")
    wcb_e = nc.dram_tensor("wcb", [128, FC], bf16, kind="ExternalInput")
    ivcol_e = nc.dram_tensor("ivcol", [128, NBLK], f32, kind="ExternalInput")
    dvcol_e = nc.dram_tensor("dvcol", [128, NBLK], f32, kind="ExternalInput")

    out_e = nc.dram_tensor("out", [NSH, FC], f32, kind="ExternalOutput")

    z0l_d = nc.dram_tensor("z0l_d", [NSH, 128], bf16)
    z0f_d = nc.dram_tensor("z0f_d", [N, 128], bf16, addr_space="Shared")
    z1l_d = nc.dram_tensor("z1l_d", [NSH, 128], bf16)
    z1f_d = nc.dram_tensor("z1f_d", [N, 128], bf16, addr_space="Shared")

    core_ids = list(range(NC_CORES))

    chunks = []
    b0 = 0
    while b0 < NBLK:
        chunks.append((b0, min(b0 + CHUNK_BLOCKS, NBLK)))
        b0 = min(b0 + CHUNK_BLOCKS, NBLK)

    with tile.TileContext(nc) as tc:
        with (
            tc.tile_pool(name="const", bufs=1) as pc,
            tc.tile_pool(name="xa", bufs=3) as px,
            tc.tile_pool(name="zb", bufs=4) as pz,
            tc.tile_pool(name="g", bufs=6) as pg,
            tc.tile_pool(name="sel", bufs=8) as psel,
            tc.tile_pool(name="psA", bufs=2, space="PSUM") as ppA,
            tc.tile_pool(name="psacc", bufs=6, space="PSUM") as ppa,
        ):
            nc.gpsimd.load_library(library_config.mlp)

            # ---- resident constants
            iota_t = pc.tile([128, 1024], bf16)
            nc.sync.dma_start(out=iota_t[:], in_=iota_e[:])
            idxA_t = pc.tile([128, TH[0] * 8], i16)
            nc.sync.dma_start(out=idxA_t[:], in_=idxA_e[:])
            idxB_t = pc.tile([128, TH[1] * 8], i16)
            nc.sync.dma_start(out=idxB_t[:], in_=idxB_e[:])
            dlA_t = pc.tile([128, TH[0]], bf16)
            nc.sync.dma_start(out=dlA_t[:], in_=dlA_e[:])
            dlB_t = pc.tile([128, TH[1]], bf16)
            nc.sync.dma_start(out=dlB_t[:], in_=dlB_e[:])
            wca_t = pc.tile([128, FC], bf16)
            nc.sync.dma_start(out=wca_t[:], in_=wca_e[:])
            wcb_t = pc.tile([128, FC], bf16)
            nc.sync.dma_start(out=wcb_t[:], in_=wcb_e[:])
            ivcol_t = pc.tile([128, NBLK], f32)
            nc.sync.dma_start(out=ivcol_t[:], in_=ivcol_e[:])
            dvcol_t = pc.tile([128, NBLK], f32)
            nc.sync.dma_start(out=dvcol_t[:], in_=dvcol_e[:])

            # ---- phase A: z0 shard = dinv * (x_shard @ Wc)
            with nc.named_scope("phaseA"):
                done = 0
                while done < NBLK:
                    nb_cnt = min(8, NBLK - done)
                    c0 = done * 128
                    cols = nb_cnt * 128
                    xa = px.tile([128, 1024], bf16, tag="xa")
                    xb = px.tile([128, 1024], bf16, tag="xb")
                    nc.sync.dma_start(
                        out=xa[:, :cols], in_=xsh_e[0:128, c0 : c0 + cols]
                    )
                    nc.scalar.dma_start(
                        out=xb[:, :cols], in_=xsh_e[128:256, c0 : c0 + cols]
                    )
                    for j in range(nb_cnt):
                        gb = done + j
                        rows = min(128, NSH - gb * 128)
                        zp = ppA.tile([128, FC], f32, space="PSUM", tag="zp")
                        nc.tensor.matmul(
                            out=zp[:],
                            lhsT=xa[:, j * 128 : (j + 1) * 128],
                            rhs=wca_t[:],
                            start=True,
                            stop=False,
                        )
                        nc.tensor.matmul(
                            out=zp[:],
                            lhsT=xb[:, j * 128 : (j + 1) * 128],
                            rhs=wcb_t[:],
                            start=False,
                            stop=True,
                        )
                        z0s = pz.tile([128, 128], bf16, tag="z0s")
                        nc.vector.memset(z0s[:, FC:128], 0.0)
                        nc.scalar.activation(
                            z0s[:, 0:FC], zp[:], COPY,
                            scale=dvcol_t[:, gb : gb + 1],
                        )
                        nc.sync.dma_start(
                            out=z0l_d[gb * 128 : gb * 128 + rows], in_=z0s[:rows]
                        )
                    done += nb_cnt

            with nc.named_scope("ag0"):
                nc.gpsimd.collective_compute(
                    "AllGather",
                    mybir.AluOpType.bypass,
                    ins=[z0l_d[:]],
                    outs=[z0f_d[:]],
                    replica_groups=[core_ids],
                )

            # ---- shared SpMM: out[dst_block] = scale * sum_e msg[e]
            GT = 7  # tiles per gather sub-call: 57 descs/engine-packet (<=64 HW cap)

            def spmm(srcA, srcB, scale_t, store, qoff=0):
                qn = qoff
                for (cb0, cb1) in chunks:
                    ctA = sum(T[b][0] for b in range(cb0, cb1))
                    ctB = sum(T[b][1] for b in range(cb0, cb1))
                    offA, offB = toff[0][cb0], toff[1][cb0]
                    gA = pg.tile([128, ctA, 128], bf16, tag="gA")
                    gB = pg.tile([128, ctB, 128], bf16, tag="gB")
                    for g, src, idx_t, off, ct in (
                        (gA, srcA, idxA_t, offA, ctA),
                        (gB, srcB, idxB_t, offB, ctB),
                    ):
                        t0 = 0
                        while t0 < ct:
                            tn = min(GT, ct - t0)
                            nc.gpsimd.dma_gather(
                                g[:, t0 : t0 + tn, :], src,
                                idx_t[:, (off + t0) * 8 : (off + t0 + tn) * 8],
                                tn * 128, tn * 128, 128,
                                single_packet=True, queue_num=qn % 4,
                            )
                            qn += 1
                            t0 += tn
                    for b in range(cb0, cb1):
                        acc = ppa.tile([128, FC], f32, space="PSUM", tag="acc")
                        n_mm = T[b][0] + T[b][1]
                        mi = 0
                        for h, g, dl_t, off0 in (
                            (0, gA, dlA_t, offA),
                            (1, gB, dlB_t, offB),
                        ):
                            tloc0 = toff[h][b] - off0
                            nt = T[b][h]
                            done = 0
                            while done < nt:
                                k = min(8, nt - done)
                                sel = psel.tile([128, k, 128], bf16, tag="sel")
                                nc.vector.tensor_tensor(
                                    out=sel[:],
                                    in0=dl_t[
                                        :, toff[h][b] + done : toff[h][b] + done + k
                                    ].to_broadcast([128, k, 128]),
                                    in1=iota_t[:, : k * 128],
                                    op=mybir.AluOpType.is_equal,
                                )
                                for q in range(k):
                                    nc.tensor.matmul(
                                        out=acc[:],
                                        lhsT=sel[:, q, :],
                                        rhs=g[:, tloc0 + done + q, 0:FC],
                                        start=(mi == 0),
                                        stop=(mi == n_mm - 1),
                                    )
                                    mi += 1
                                done += k
                        store(b, acc, scale_t)

            # ---- SpMM1 -> z1 table shard; AllGather
            def store_z1(b, acc, scale_t):
                rows = min(128, NSH - b * 128)
                z1s = pz.tile([128, 128], bf16, tag="z1s")
                nc.vector.memset(z1s[:, FC:128], 0.0)
                nc.scalar.activation(
                    z1s[:, 0:FC], acc[:], COPY, scale=scale_t[:, b : b + 1]
                )
                nc.sync.dma_start(
                    out=z1l_d[b * 128 : b * 128 + rows], in_=z1s[:rows]
                )

            with nc.named_scope("spmm1"):
                spmm(z0f_d[0:HALF], z0f_d[HALF:N], ivcol_t, store_z1, qoff=0)

            with nc.named_scope("ag1"):
                nc.gpsimd.collective_compute(
                    "AllGather",
                    mybir.AluOpType.bypass,
                    ins=[z1l_d[:]],
                    outs=[z1f_d[:]],
                    replica_groups=[core_ids],
                )

            # ---- SpMM2 -> final output
            def store_out(b, acc, scale_t):
                rows = min(128, NSH - b * 128)
                os_ = pz.tile([128, FC], f32, tag="outs")
                nc.scalar.activation(
                    os_[:], acc[:], COPY, scale=scale_t[:, b : b + 1]
                )
                nc.sync.dma_start(
                    out=out_e[b * 128 : b * 128 + rows], in_=os_[:rows]
                )

            with nc.named_scope("spmm2"):
                spmm(z1f_d[0:HALF], z1f_d[HALF:N], dvcol_t, store_out, qoff=2)

    nc.compile()
    return nc


_CACHE = {}


def kernel(**inputs):
    in_maps, plan = _prep(**inputs)
    key = tuple(tuple(t) for t in plan["T"])
    if key not in _CACHE:
        _CACHE[key] = _build(plan)
    nc = _CACHE[key]
    res = run_bass_kernel_spmd(nc, in_maps, list(range(NC_CORES)))
    out = np.concatenate(
        [res.results[c]["out"] for c in range(NC_CORES)], axis=0
    )
    if plan["bias_corr"] is not None:
        out = out + plan["bias_corr"]
    mu = np.ascontiguousarray(out[:, :OUT_D])
    lv = np.ascontiguousarray(out[:, OUT_D:])
    return (mu, lv)



# revision 7
# speedup vs baseline: 1.1787x; 1.0070x over previous
"""Two-layer GCN encoder (GCNConv x2 -> mu/logvar heads) on 8 TRN2 NeuronCores.

v2: the module is linear (no activation between convs), so it collapses to
  [mu | lv] = A^2 @ X @ Wc,   Wc = W1 @ W2 @ [W_mu | W_lv]  (256 x 64)
with A = D^-1/2 (Adj + I) D^-1/2. Folding the normalization row-wise:
  z0 = dinv * (X @ Wc)          (sharded: each core computes its 6250 rows)
  z1 = invdeg * (Ahat @ z0)     (SpMM over edges incl self-loops)
  Y  = dinv * (Ahat @ z1)       (second SpMM; Y[:, :32] = mu, Y[:, 32:] = lv)
Nonzero biases are corrected host-side via the rank-1 identity
  Y += s * (b1 W2 Wh) + (b2 Wh + bh),  s = A @ 1.

Device structure:
  - z0/z1 tables are [N, 128] bf16 (cols 0:64 = data, rest zero) so gather
    rows stay 256B; tables are exchanged with AllGather after each stage.
  - Edges partitioned by destination core, sorted by (src-half, dst);
    messages fetched with dma_gather (int16 idx -> <=32767-row halves) on
    4 SWDGE queues; scatter-add into PSUM via one-hot matmul with the
    one-hot as lhsT: acc[dst, f] += sel^T @ msg, so PSUM is node-major and
    feeds the next table (or the output) directly after a per-row scale.
"""

import os

import ml_dtypes
import numpy as np

import concourse.bacc as bacc
import concourse.bass as bass
import concourse.mybir as mybir
import concourse.tile as tile
from concourse import library_config
from concourse.bass_utils import run_bass_kernel_spmd

# ---- problem constants (hardcoded per harness contract) ----
N = 50000
IN_D, HID1, HID2, OUT_D = 256, 128, 64, 32
NC_CORES = 8
NSH = N // NC_CORES  # 6250 dst nodes per core
NBLK = (NSH + 127) // 128  # 49 dst blocks per core
HALF = 24960  # block-aligned split; both halves < 32767 (int16 gather idx)
NBLK_ALL = -(-N // 128)  # 391 node blocks total
CHUNK_BLOCKS = 3  # dst blocks per gather chunk
FC = 64  # collapsed feature count

BF16 = ml_dtypes.bfloat16

_tile_patched = False


def _patch_tile_drain():
    """walrus in this env rejects >~2 sem waits on one instruction; Tile's
    kernel-tail drain aggregates one wait per live semaphore. Move the excess
    onto dedicated single-wait SP nops that precede the drain."""
    global _tile_patched
    if _tile_patched:
        return
    _tile_patched = True
    _orig = tile.TileContext._drain_and_barrier

    def _patched(self, tick_clock, wait_clock):
        nc = self.nc
        nops = [nc.sync.nop(nofuse=True, hint=f"dw_{i}").ins for i in range(64)]
        _orig(self, tick_clock, wait_clock)
        ni = 0
        for inst in nc.cur_bb.bb.instructions:
            if "Drain" not in type(inst).__name__:
                continue
            ow = inst.sync_info.on_wait if inst.sync_info else []
            if len(ow) > 1:
                waits = list(ow)
                for w in waits[:-1]:
                    nops[ni].sync_info = mybir.SyncInfo(on_wait=[w], on_update=[])
                    ni += 1
                inst.sync_info.on_wait[:] = waits[-1:]

    tile.TileContext._drain_and_barrier = _patched


def _prep(x, edge_index, W1, b1, W2, b2, W_mu, b_mu, W_lv, b_lv):
    """Host-side graph partitioning + input staging. Returns (in_maps, plan)."""
    src = np.asarray(edge_index[0], dtype=np.int64)
    dst = np.asarray(edge_index[1], dtype=np.int64)
    loop = np.arange(N, dtype=np.int64)
    src_a = np.concatenate([src, loop])
    dst_a = np.concatenate([dst, loop])

    deg = np.bincount(dst_a, minlength=N).astype(np.float64)
    dinv = deg**-0.5
    invdeg = 1.0 / deg

    # sort edges by (src-half, dst): each (dst-block, half) group contiguous
    half = (src_a >= HALF).astype(np.int64)
    key = half * N + dst_a
    order = np.argsort(key, kind="stable")
    s_sorted = src_a[order]
    d_sorted = dst_a[order]
    bnd = np.searchsorted(key[order], np.arange(2 * N + 1))

    # per-(core, block, half) counts -> core-independent tile counts
    T = [[0, 0] for _ in range(NBLK)]
    counts = np.zeros((NC_CORES, NBLK, 2), dtype=np.int64)
    for c in range(NC_CORES):
        for b in range(NBLK):
            lo = c * NSH + b * 128
            hi = min(c * NSH + (b + 1) * 128, (c + 1) * NSH)
            for h in range(2):
                counts[c, b, h] = bnd[h * N + hi] - bnd[h * N + lo]
    for b in range(NBLK):
        for h in range(2):
            T[b][h] = max(1, int(-(-counts[:, b, h].max() // 128)))

    TH = [sum(T[b][h] for b in range(NBLK)) for h in range(2)]
    toff = [[0] * NBLK, [0] * NBLK]
    for h in range(2):
        acc = 0
        for b in range(NBLK):
            toff[h][b] = acc
            acc += T[b][h]

    # per-core padded idx / dstloc streams
    core_data = []
    for c in range(NC_CORES):
        idx_streams = []
        dl_streams = []
        for h in range(2):
            idx = np.zeros(TH[h] * 128, dtype=np.int16)
            dl = np.full(TH[h] * 128, -1.0, dtype=np.float32)
            for b in range(NBLK):
                lo = c * NSH + b * 128
                hi = min(c * NSH + (b + 1) * 128, (c + 1) * NSH)
                e0, e1 = bnd[h * N + lo], bnd[h * N + hi]
                cnt = e1 - e0
                off = toff[h][b] * 128
                idx[off : off + cnt] = (s_sorted[e0:e1] - h * HALF).astype(np.int16)
                dl[off : off + cnt] = (d_sorted[e0:e1] - lo).astype(np.float32)
            packed = np.tile(np.ascontiguousarray(idx.reshape(-1, 16).T), (8, 1))
            idx_streams.append(packed)
            dl_streams.append(np.ascontiguousarray(dl.reshape(-1, 128).T).astype(BF16))
        core_data.append((idx_streams, dl_streams))

    # collapsed weights
    W1_ = np.asarray(W1, np.float64)
    W2_ = np.asarray(W2, np.float64)
    Wh = np.concatenate(
        [np.asarray(W_mu, np.float64), np.asarray(W_lv, np.float64)], axis=1
    )  # [64, 64]
    Wc = W1_ @ W2_ @ Wh  # [256, 64]
    wca = Wc[:128].astype(BF16)
    wcb = Wc[128:].astype(BF16)

    # host-side bias correction (zero for this module)
    r1 = (np.asarray(b1, np.float64) @ W2_) @ Wh  # [64]
    r0 = np.asarray(b2, np.float64) @ Wh + np.concatenate(
        [np.asarray(b_mu, np.float64), np.asarray(b_lv, np.float64)]
    )
    if np.any(r1) or np.any(r0):
        s_vec = dinv * np.bincount(dst_a, weights=dinv[src_a], minlength=N)
        bias_corr = (s_vec[:, None] * r1[None, :] + r0[None, :]).astype(np.float32)
    else:
        bias_corr = None

    iota_rep = np.tile(np.arange(128, dtype=np.float32), (128, 8)).astype(BF16)

    xf = np.asarray(x, np.float32)
    in_maps = []
    for c in range(NC_CORES):
        (idxA, idxB), (dlA, dlB) = core_data[c]
        own = slice(c * NSH, (c + 1) * NSH)
        xsh = np.zeros((IN_D, NBLK * 128), np.float32)
        xsh[:, :NSH] = xf[own].T
        tmp_iv = np.zeros(NBLK * 128, np.float64)
        tmp_dv = np.zeros(NBLK * 128, np.float64)
        tmp_iv[:NSH] = invdeg[own]
        tmp_dv[:NSH] = dinv[own]
        in_maps.append(
            {
                "xsh": xsh.astype(BF16),
                "iota": iota_rep,
                "idxA": idxA,
                "idxB": idxB,
                "dlA": dlA,
                "dlB": dlB,
                "wca": wca,
                "wcb": wcb,
                "ivcol": np.ascontiguousarray(
                    tmp_iv.reshape(NBLK, 128).T
                ).astype(np.float32),
                "dvcol": np.ascontiguousarray(
                    tmp_dv.reshape(NBLK, 128).T
                ).astype(np.float32),
            }
        )

    plan = {"T": T, "TH": TH, "toff": toff, "bias_corr": bias_corr}
    return in_maps, plan


def _build(plan):
    _patch_tile_drain()
    T, TH, toff = plan["T"], plan["TH"], plan["toff"]

    nc = bacc.Bacc("TRN2", num_swdge_queues=4, dynamic_dma_scratch_size=49152)
    f32, bf16, i16 = mybir.dt.float32, mybir.dt.bfloat16, mybir.dt.int16
    COPY = mybir.ActivationFunctionType.Copy

    xsh_e = nc.dram_tensor("xsh", [IN_D, NBLK * 128], bf16, kind="ExternalInput")
    iota_e = nc.dram_tensor("iota", [128, 1024], bf16, kind="ExternalInput")
    idxA_e = nc.dram_tensor("idxA", [128, TH[0] * 8], i16, kind="ExternalInput")
    idxB_e = nc.dram_tensor("idxB", [128, TH[1] * 8], i16, kind="ExternalInput")
    dlA_e = nc.dram_tensor("dlA", [128, TH[0]], bf16, kind="ExternalInput")
    dlB_e = nc.dram_tensor("dlB", [128, TH[1]], bf16, kind="ExternalInput")
    wca_e = nc.dram_tensor("wca", [128, FC], bf16, kind="ExternalInput")
    wcb_e = nc.dram_tensor("wcb", [128, FC], bf16, kind="ExternalInput")
    ivcol_e = nc.dram_tensor("ivcol", [128, NBLK], f32, kind="ExternalInput")
    dvcol_e = nc.dram_tensor("dvcol", [128, NBLK], f32, kind="ExternalInput")

    out_e = nc.dram_tensor("out", [NSH, FC], f32, kind="ExternalOutput")

    z0l_d = nc.dram_tensor("z0l_d", [NSH, 128], bf16)
    z0f_d = nc.dram_tensor("z0f_d", [N, 128], bf16, addr_space="Shared")
    z1l_d = nc.dram_tensor("z1l_d", [NSH, 128], bf16)
    z1f_d = nc.dram_tensor("z1f_d", [N, 128], bf16, addr_space="Shared")

    core_ids = list(range(NC_CORES))

    chunks = []
    b0 = 0
    while b0 < NBLK:
        chunks.append((b0, min(b0 + CHUNK_BLOCKS, NBLK)))
        b0 = min(b0 + CHUNK_BLOCKS, NBLK)

    with tile.TileContext(nc) as tc:
        with (
            tc.tile_pool(name="const", bufs=1) as pc,
            tc.tile_pool(name="xa", bufs=3) as px,
            tc.tile_pool(name="zb", bufs=4) as pz,
            tc.tile_pool(name="g", bufs=6) as pg,
            tc.tile_pool(name="sel", bufs=8) as psel,
            tc.tile_pool(name="psA", bufs=2, space="PSUM") as ppA,
            tc.tile_pool(name="psacc", bufs=6, space="PSUM") as ppa,
        ):
            nc.gpsimd.load_library(library_config.mlp)

            # ---- resident constants
            iota_t = pc.tile([128, 1024], bf16)
            nc.sync.dma_start(out=iota_t[:], in_=iota_e[:])
            idxA_t = pc.tile([128, TH[0] * 8], i16)
            nc.sync.dma_start(out=idxA_t[:], in_=idxA_e[:])
            idxB_t = pc.tile([128, TH[1] * 8], i16)
            nc.sync.dma_start(out=idxB_t[:], in_=idxB_e[:])
            dlA_t = pc.tile([128, TH[0]], bf16)
            nc.sync.dma_start(out=dlA_t[:], in_=dlA_e[:])
            dlB_t = pc.tile([128, TH[1]], bf16)
            nc.sync.dma_start(out=dlB_t[:], in_=dlB_e[:])
            wca_t = pc.tile([128, FC], bf16)
            nc.sync.dma_start(out=wca_t[:], in_=wca_e[:])
            wcb_t = pc.tile([128, FC], bf16)
            nc.sync.dma_start(out=wcb_t[:], in_=wcb_e[:])
            ivcol_t = pc.tile([128, NBLK], f32)
            nc.sync.dma_start(out=ivcol_t[:], in_=ivcol_e[:])
            dvcol_t = pc.tile([128, NBLK], f32)
            nc.sync.dma_start(out=dvcol_t[:], in_=dvcol_e[:])

            # ---- phase A: z0 shard = dinv * (x_shard @ Wc)
            with nc.named_scope("phaseA"):
                done = 0
                while done < NBLK:
                    nb_cnt = min(8, NBLK - done)
                    c0 = done * 128
                    cols = nb_cnt * 128
                    xa = px.tile([128, 1024], bf16, tag="xa")
                    xb = px.tile([128, 1024], bf16, tag="xb")
                    nc.sync.dma_start(
                        out=xa[:, :cols], in_=xsh_e[0:128, c0 : c0 + cols]
                    )
                    nc.scalar.dma_start(
                        out=xb[:, :cols], in_=xsh_e[128:256, c0 : c0 + cols]
                    )
                    for j in range(nb_cnt):
                        gb = done + j
                        rows = min(128, NSH - gb * 128)
                        zp = ppA.tile([128, FC], f32, space="PSUM", tag="zp")
                        nc.tensor.matmul(
                            out=zp[:],
                            lhsT=xa[:, j * 128 : (j + 1) * 128],
                            rhs=wca_t[:],
                            start=True,
                            stop=False,
                        )
                        nc.tensor.matmul(
                            out=zp[:],
                            lhsT=xb[:, j * 128 : (j + 1) * 128],
                            rhs=wcb_t[:],
                            start=False,
                            stop=True,
                        )
                        z0s = pz.tile([128, 128], bf16, tag="z0s")
                        nc.vector.memset(z0s[:, FC:128], 0.0)
                        nc.scalar.activation(
                            z0s[:, 0:FC], zp[:], COPY,
                            scale=dvcol_t[:, gb : gb + 1],
                        )
                        nc.sync.dma_start(
                            out=z0l_d[gb * 128 : gb * 128 + rows], in_=z0s[:rows]
                        )
                    done += nb_cnt

            with nc.named_scope("ag0"):
                nc.gpsimd.collective_compute(
                    "AllGather",
                    mybir.AluOpType.bypass,
                    ins=[z0l_d[:]],
                    outs=[z0f_d[:]],
                    replica_groups=[core_ids],
                )

            # ---- shared SpMM: out[dst_block] = scale * sum_e msg[e]
            GT = 7  # tiles per gather sub-call: 57 descs/engine-packet (<=64 HW cap)

            def spmm(srcA, srcB, scale_t, store, qoff=0):
                qn = qoff
                for (cb0, cb1) in chunks:
                    ctA = sum(T[b][0] for b in range(cb0, cb1))
                    ctB = sum(T[b][1] for b in range(cb0, cb1))
                    offA, offB = toff[0][cb0], toff[1][cb0]
                    gA = pg.tile([128, ctA, 128], bf16, tag="gA")
                    gB = pg.tile([128, ctB, 128], bf16, tag="gB")
                    for g, src, idx_t, off, ct in (
                        (gA, srcA, idxA_t, offA, ctA),
                        (gB, srcB, idxB_t, offB, ctB),
                    ):
                        t0 = 0
                        while t0 < ct:
                            tn = min(GT, ct - t0)
                            nc.gpsimd.dma_gather(
                                g[:, t0 : t0 + tn, :], src,
                                idx_t[:, (off + t0) * 8 : (off + t0 + tn) * 8],
                                tn * 128, tn * 128, 128,
                                single_packet=True, queue_num=qn % 4,
                            )
                            qn += 1
                            t0 += tn
                    for b in range(cb0, cb1):
                        acc = ppa.tile([128, FC], f32, space="PSUM", tag="acc")
                        n_mm = T[b][0] + T[b][1]
                        mi = 0
                        for h, g, dl_t, off0 in (
                            (0, gA, dlA_t, offA),
                            (1, gB, dlB_t, offB),
                        ):
                            tloc0 = toff[h][b] - off0
                            nt = T[b][h]
                            done = 0
                            while done < nt:
                                k = min(8, nt - done)
                                sel = psel.tile([128, k, 128], bf16, tag="sel")
                                nc.vector.tensor_tensor(
                                    out=sel[:],
                                    in0=dl_t[
                                        :, toff[h][b] + done : toff[h][b] + done + k
                                    ].to_broadcast([128, k, 128]),
                                    in1=iota_t[:, : k * 128],
                                    op=mybir.AluOpType.is_equal,
                                )
                                for q in range(k):
                                    nc.tensor.matmul(
                                        out=acc[:],
                                        lhsT=sel[:, q, :],
                                        rhs=g[:, tloc0 + done + q, 0:FC],
                                        start=(mi == 0),
                                        stop=(mi == n_mm - 1),
                                    )
                                    mi += 1
                                done += k
                        store(b, acc, scale_t)

            # ---- SpMM1 -> z1 table shard; AllGather
            def store_z1(b, acc, scale_t):
                rows = min(128, NSH - b * 128)
                z1s = pz.tile([128, 128], bf16, tag="z1s")
                nc.vector.memset(z1s[:, FC:128], 0.0)
                nc.scalar.activation(
                    z1s[:, 0:FC], acc[:], COPY, scale=scale_t[:, b : b + 1]
                )
                nc.sync.dma_start(
                    out=z1l_d[b * 128 : b * 128 + rows], in_=z1s[:rows]
                )

            with nc.named_scope("spmm1"):
                spmm(z0f_d[0:HALF], z0f_d[HALF:N], ivcol_t, store_z1, qoff=0)

            with nc.named_scope("ag1"):
                nc.gpsimd.collective_compute(
                    "AllGather",
                    mybir.AluOpType.bypass,
                    ins=[z1l_d[:]],
                    outs=[z1f_d[:]],
                    replica_groups=[core_ids],
                )

            # ---- SpMM2 -> final output
            def store_out(b, acc, scale_t):
                rows = min(128, NSH - b * 128)
                os_ = pz.tile([128, FC], f32, tag="outs")
                nc.scalar.activation(
                    os_[:], acc[:], COPY, scale=scale_t[:, b : b + 1]
                )
                nc.sync.dma_start(
                    out=out_e[b * 128 : b * 128 + rows], in_=os_[:rows]
                )

            with nc.named_scope("spmm2"):
                spmm(z1f_d[0:HALF], z1f_d[HALF:N], dvcol_t, store_out, qoff=2)

    nc.compile()
    return nc


_CACHE = {}


def kernel(**inputs):
    in_maps, plan = _prep(**inputs)
    key = tuple(tuple(t) for t in plan["T"])
    if key not in _CACHE:
        _CACHE[key] = _build(plan)
    nc = _CACHE[key]
    res = run_bass_kernel_spmd(nc, in_maps, list(range(NC_CORES)))
    out = np.concatenate(
        [res.results[c]["out"] for c in range(NC_CORES)], axis=0
    )
    if plan["bias_corr"] is not None:
        out = out + plan["bias_corr"]
    mu = np.ascontiguousarray(out[:, :OUT_D])
    lv = np.ascontiguousarray(out[:, OUT_D:])
    return (mu, lv)



# revision 9
# speedup vs baseline: 1.2145x; 1.0304x over previous
"""Two-layer GCN encoder (GCNConv x2 -> mu/logvar heads) on 8 TRN2 NeuronCores.

v3: linear module collapses to [mu | lv] = A^2 @ X @ Wc (Wc = W1 W2 [W_mu|W_lv]).
Folding normalization row-wise (z0 = dinv*(X@Wc), z1 = invdeg*(Ahat z0),
Y = dinv*(Ahat z1)) with Ahat = Adj + I handled as:
  - real edges via dma_gather + one-hot-matmul scatter into PSUM
  - self loops via an identity matmul on the SBUF-resident local z blocks

Device structure (v3):
  - z tables are UNPADDED [N, 64] bf16; the gather views them as pair rows
    [N/2, 128] (256B rows, 2 nodes each). Edge streams are split by src
    parity: even-src edges use gathered cols 0:64, odd-src cols 64:128.
  - AllGather moves 6.4MB tables (halved vs padded layout).
  - Gather sub-calls of <=7 tiles (57 descs/engine packet, under the 64-desc
    HW packet cap) with single_packet=True, rotating 4 SWDGE queues;
    3x descriptor-ring depth (dynamic_dma_scratch_size=49152).
  - scatter-add into PSUM via one-hot matmul (sel as lhsT), per-row scale on
    the scalar engine, local z blocks kept in SBUF for the self-loop term.
"""

import os

import ml_dtypes
import numpy as np

import concourse.bacc as bacc
import concourse.bass as bass
import concourse.mybir as mybir
import concourse.tile as tile
from concourse import library_config
from concourse.bass_utils import run_bass_kernel_spmd

# ---- problem constants (hardcoded per harness contract) ----
N = 50000
IN_D, HID1, HID2, OUT_D = 256, 128, 64, 32
NC_CORES = 8
NSH = N // NC_CORES  # 6250 dst nodes per core
NBLK = (NSH + 127) // 128  # 49 dst blocks per core
NPAIR = N // 2  # pair rows in the gather view (int16-safe: 25000 < 32767)
CHUNK_BLOCKS = 3  # dst blocks per gather chunk
FC = 64  # collapsed feature count
GT = 7  # tiles per gather sub-call: 57 descs/engine packet (<=64 HW cap)

BF16 = ml_dtypes.bfloat16

_tile_patched = False


def _patch_tile_drain():
    """walrus in this env rejects >~2 sem waits on one instruction; Tile's
    kernel-tail drain aggregates one wait per live semaphore. Move the excess
    onto dedicated single-wait SP nops that precede the drain."""
    global _tile_patched
    if _tile_patched:
        return
    _tile_patched = True
    _orig = tile.TileContext._drain_and_barrier

    def _patched(self, tick_clock, wait_clock):
        nc = self.nc
        nops = [nc.sync.nop(nofuse=True, hint=f"dw_{i}").ins for i in range(64)]
        _orig(self, tick_clock, wait_clock)
        ni = 0
        for inst in nc.cur_bb.bb.instructions:
            if "Drain" not in type(inst).__name__:
                continue
            ow = inst.sync_info.on_wait if inst.sync_info else []
            if len(ow) > 1:
                waits = list(ow)
                for w in waits[:-1]:
                    nops[ni].sync_info = mybir.SyncInfo(on_wait=[w], on_update=[])
                    ni += 1
                inst.sync_info.on_wait[:] = waits[-1:]

    tile.TileContext._drain_and_barrier = _patched


def _prep(x, edge_index, W1, b1, W2, b2, W_mu, b_mu, W_lv, b_lv):
    """Host-side graph partitioning + input staging. Returns (in_maps, plan)."""
    src = np.asarray(edge_index[0], dtype=np.int64)
    dst = np.asarray(edge_index[1], dtype=np.int64)

    # degrees include the self loop (handled on-device via identity matmul)
    deg = (np.bincount(dst, minlength=N) + 1).astype(np.float64)
    dinv = deg**-0.5
    invdeg = 1.0 / deg

    # sort real edges by (src-parity, dst): each (dst-block, parity) group
    # contiguous; parity selects gathered cols 0:64 vs 64:128 of a pair row
    par = src % 2
    key = par * N + dst
    order = np.argsort(key, kind="stable")
    s_sorted = src[order]
    d_sorted = dst[order]
    bnd = np.searchsorted(key[order], np.arange(2 * N + 1))

    # per-(core, block, parity) counts -> core-independent tile counts
    T = [[0, 0] for _ in range(NBLK)]
    counts = np.zeros((NC_CORES, NBLK, 2), dtype=np.int64)
    for c in range(NC_CORES):
        for b in range(NBLK):
            lo = c * NSH + b * 128
            hi = min(c * NSH + (b + 1) * 128, (c + 1) * NSH)
            for h in range(2):
                counts[c, b, h] = bnd[h * N + hi] - bnd[h * N + lo]
    for b in range(NBLK):
        for h in range(2):
            T[b][h] = max(1, int(-(-counts[:, b, h].max() // 128)))

    TH = [sum(T[b][h] for b in range(NBLK)) for h in range(2)]
    toff = [[0] * NBLK, [0] * NBLK]
    for h in range(2):
        acc = 0
        for b in range(NBLK):
            toff[h][b] = acc
            acc += T[b][h]

    # per-core padded idx / dstloc streams (idx = pair row = src // 2)
    core_data = []
    for c in range(NC_CORES):
        idx_streams = []
        dl_streams = []
        for h in range(2):
            idx = np.zeros(TH[h] * 128, dtype=np.int16)
            dl = np.full(TH[h] * 128, -1.0, dtype=np.float32)
            for b in range(NBLK):
                lo = c * NSH + b * 128
                hi = min(c * NSH + (b + 1) * 128, (c + 1) * NSH)
                e0, e1 = bnd[h * N + lo], bnd[h * N + hi]
                cnt = e1 - e0
                off = toff[h][b] * 128
                idx[off : off + cnt] = (s_sorted[e0:e1] // 2).astype(np.int16)
                dl[off : off + cnt] = (d_sorted[e0:e1] - lo).astype(np.float32)
            packed = np.tile(np.ascontiguousarray(idx.reshape(-1, 16).T), (8, 1))
            idx_streams.append(packed)
            dl_streams.append(np.ascontiguousarray(dl.reshape(-1, 128).T).astype(BF16))
        core_data.append((idx_streams, dl_streams))

    # collapsed weights
    W1_ = np.asarray(W1, np.float64)
    W2_ = np.asarray(W2, np.float64)
    Wh = np.concatenate(
        [np.asarray(W_mu, np.float64), np.asarray(W_lv, np.float64)], axis=1
    )  # [64, 64]
    Wc = W1_ @ W2_ @ Wh  # [256, 64]
    wca = Wc[:128].astype(BF16)
    wcb = Wc[128:].astype(BF16)

    # host-side bias correction (zero for this module)
    r1 = (np.asarray(b1, np.float64) @ W2_) @ Wh  # [64]
    r0 = np.asarray(b2, np.float64) @ Wh + np.concatenate(
        [np.asarray(b_mu, np.float64), np.asarray(b_lv, np.float64)]
    )
    if np.any(r1) or np.any(r0):
        s_vec = dinv * (
            np.bincount(dst, weights=dinv[src], minlength=N) + dinv
        )
        bias_corr = (s_vec[:, None] * r1[None, :] + r0[None, :]).astype(np.float32)
    else:
        bias_corr = None

    iota_rep = np.tile(np.arange(128, dtype=np.float32), (128, 8)).astype(BF16)
    ident = np.eye(128, dtype=np.float32).astype(BF16)

    xf = np.asarray(x, np.float32)
    in_maps = []
    for c in range(NC_CORES):
        (idxA, idxB), (dlA, dlB) = core_data[c]
        own = slice(c * NSH, (c + 1) * NSH)
        xsh = np.zeros((IN_D, NBLK * 128), np.float32)
        xsh[:, :NSH] = xf[own].T
        tmp_iv = np.zeros(NBLK * 128, np.float64)
        tmp_dv = np.zeros(NBLK * 128, np.float64)
        tmp_iv[:NSH] = invdeg[own]
        tmp_dv[:NSH] = dinv[own]
        in_maps.append(
            {
                "xsh": xsh.astype(BF16),
                "iota": iota_rep,
                "ident": ident,
                "idxA": idxA,
                "idxB": idxB,
                "dlA": dlA,
                "dlB": dlB,
                "wca": wca,
                "wcb": wcb,
                "ivcol": np.ascontiguousarray(
                    tmp_iv.reshape(NBLK, 128).T
                ).astype(np.float32),
                "dvcol": np.ascontiguousarray(
                    tmp_dv.reshape(NBLK, 128).T
                ).astype(np.float32),
            }
        )

    plan = {"T": T, "TH": TH, "toff": toff, "bias_corr": bias_corr}
    return in_maps, plan


def _build(plan):
    _patch_tile_drain()
    T, TH, toff = plan["T"], plan["TH"], plan["toff"]

    nc = bacc.Bacc("TRN2", num_swdge_queues=4, dynamic_dma_scratch_size=49152)
    f32, bf16, i16 = mybir.dt.float32, mybir.dt.bfloat16, mybir.dt.int16
    COPY = mybir.ActivationFunctionType.Copy

    xsh_e = nc.dram_tensor("xsh", [IN_D, NBLK * 128], bf16, kind="ExternalInput")
    iota_e = nc.dram_tensor("iota", [128, 1024], bf16, kind="ExternalInput")
    ident_e = nc.dram_tensor("ident", [128, 128], bf16, kind="ExternalInput")
    idxA_e = nc.dram_tensor("idxA", [128, TH[0] * 8], i16, kind="ExternalInput")
    idxB_e = nc.dram_tensor("idxB", [128, TH[1] * 8], i16, kind="ExternalInput")
    dlA_e = nc.dram_tensor("dlA", [128, TH[0]], bf16, kind="ExternalInput")
    dlB_e = nc.dram_tensor("dlB", [128, TH[1]], bf16, kind="ExternalInput")
    wca_e = nc.dram_tensor("wca", [128, FC], bf16, kind="ExternalInput")
    wcb_e = nc.dram_tensor("wcb", [128, FC], bf16, kind="ExternalInput")
    ivcol_e = nc.dram_tensor("ivcol", [128, NBLK], f32, kind="ExternalInput")
    dvcol_e = nc.dram_tensor("dvcol", [128, NBLK], f32, kind="ExternalInput")

    out_e = nc.dram_tensor("out", [NSH, FC], f32, kind="ExternalOutput")

    z0l_d = nc.dram_tensor("z0l_d", [NSH, FC], bf16)
    z0f_d = nc.dram_tensor("z0f_d", [NPAIR, 2 * FC], bf16, addr_space="Shared")
    z1l_d = nc.dram_tensor("z1l_d", [NSH, FC], bf16)
    z1f_d = nc.dram_tensor("z1f_d", [NPAIR, 2 * FC], bf16, addr_space="Shared")

    core_ids = list(range(NC_CORES))

    chunks = []
    b0 = 0
    while b0 < NBLK:
        chunks.append((b0, min(b0 + CHUNK_BLOCKS, NBLK)))
        b0 = min(b0 + CHUNK_BLOCKS, NBLK)

    with tile.TileContext(nc) as tc:
        with (
            tc.tile_pool(name="const", bufs=1) as pc,
            tc.tile_pool(name="xa", bufs=3) as px,
            tc.tile_pool(name="zl0", bufs=NBLK) as pzl0,
            tc.tile_pool(name="zl1", bufs=NBLK) as pzl1,
            tc.tile_pool(name="zb", bufs=4) as pz,
            tc.tile_pool(name="g", bufs=6) as pg,
            tc.tile_pool(name="sel", bufs=8) as psel,
            tc.tile_pool(name="psA", bufs=2, space="PSUM") as ppA,
            tc.tile_pool(name="psacc", bufs=6, space="PSUM") as ppa,
        ):
            nc.gpsimd.load_library(library_config.mlp)

            # ---- resident constants
            iota_t = pc.tile([128, 1024], bf16)
            nc.sync.dma_start(out=iota_t[:], in_=iota_e[:])
            ident_t = pc.tile([128, 128], bf16)
            nc.sync.dma_start(out=ident_t[:], in_=ident_e[:])
            idxA_t = pc.tile([128, TH[0] * 8], i16)
            nc.sync.dma_start(out=idxA_t[:], in_=idxA_e[:])
            idxB_t = pc.tile([128, TH[1] * 8], i16)
            nc.sync.dma_start(out=idxB_t[:], in_=idxB_e[:])
            dlA_t = pc.tile([128, TH[0]], bf16)
            nc.sync.dma_start(out=dlA_t[:], in_=dlA_e[:])
            dlB_t = pc.tile([128, TH[1]], bf16)
            nc.sync.dma_start(out=dlB_t[:], in_=dlB_e[:])
            wca_t = pc.tile([128, FC], bf16)
            nc.sync.dma_start(out=wca_t[:], in_=wca_e[:])
            wcb_t = pc.tile([128, FC], bf16)
            nc.sync.dma_start(out=wcb_t[:], in_=wcb_e[:])
            ivcol_t = pc.tile([128, NBLK], f32)
            nc.sync.dma_start(out=ivcol_t[:], in_=ivcol_e[:])
            dvcol_t = pc.tile([128, NBLK], f32)
            nc.sync.dma_start(out=dvcol_t[:], in_=dvcol_e[:])

            z0loc = [None] * NBLK
            z1loc = [None] * NBLK

            # ---- phase A: z0 shard = dinv * (x_shard @ Wc)
            with nc.named_scope("phaseA"):
                done = 0
                while done < NBLK:
                    nb_cnt = min(8, NBLK - done)
                    c0 = done * 128
                    cols = nb_cnt * 128
                    xa = px.tile([128, 1024], bf16, tag="xa")
                    xb = px.tile([128, 1024], bf16, tag="xb")
                    nc.sync.dma_start(
                        out=xa[:, :cols], in_=xsh_e[0:128, c0 : c0 + cols]
                    )
                    nc.scalar.dma_start(
                        out=xb[:, :cols], in_=xsh_e[128:256, c0 : c0 + cols]
                    )
                    for j in range(nb_cnt):
                        gb = done + j
                        rows = min(128, NSH - gb * 128)
                        zp = ppA.tile([128, FC], f32, space="PSUM", tag="zp")
                        nc.tensor.matmul(
                            out=zp[:],
                            lhsT=xa[:, j * 128 : (j + 1) * 128],
                            rhs=wca_t[:],
                            start=True,
                            stop=False,
                        )
                        nc.tensor.matmul(
                            out=zp[:],
                            lhsT=xb[:, j * 128 : (j + 1) * 128],
                            rhs=wcb_t[:],
                            start=False,
                            stop=True,
                        )
                        z0s = pzl0.tile([128, FC], bf16, tag="z0s")
                        z0loc[gb] = z0s
                        nc.scalar.activation(
                            z0s[:], zp[:], COPY,
                            scale=dvcol_t[:, gb : gb + 1],
                        )
                        nc.sync.dma_start(
                            out=z0l_d[gb * 128 : gb * 128 + rows], in_=z0s[:rows]
                        )
                    done += nb_cnt

            with nc.named_scope("ag0"):
                nc.gpsimd.collective_compute(
                    "AllGather",
                    mybir.AluOpType.bypass,
                    ins=[z0l_d[:]],
                    outs=[z0f_d[:]],
                    replica_groups=[core_ids],
                )

            # ---- shared SpMM: acc[dst_block] = z_self[block] + sum_e msg[e]
            def spmm(src_d, zloc, scale_t, store, qoff=0):
                qn = qoff
                for (cb0, cb1) in chunks:
                    ctA = sum(T[b][0] for b in range(cb0, cb1))
                    ctB = sum(T[b][1] for b in range(cb0, cb1))
                    offA, offB = toff[0][cb0], toff[1][cb0]
                    gA = pg.tile([128, ctA, 128], bf16, tag="gA")
                    gB = pg.tile([128, ctB, 128], bf16, tag="gB")
                    for g, idx_t, off, ct in (
                        (gA, idxA_t, offA, ctA),
                        (gB, idxB_t, offB, ctB),
                    ):
                        t0 = 0
                        while t0 < ct:
                            tn = min(GT, ct - t0)
                            nc.gpsimd.dma_gather(
                                g[:, t0 : t0 + tn, :], src_d,
                                idx_t[:, (off + t0) * 8 : (off + t0 + tn) * 8],
                                tn * 128, tn * 128, 128,
                                single_packet=True, queue_num=qn % 4,
                            )
                            qn += 1
                            t0 += tn
                    for b in range(cb0, cb1):
                        acc = ppa.tile([128, FC], f32, space="PSUM", tag="acc")
                        n_mm = 1 + T[b][0] + T[b][1]
                        nc.tensor.matmul(
                            out=acc[:],
                            lhsT=ident_t[:],
                            rhs=zloc[b][:],
                            start=True,
                            stop=False,
                        )
                        mi = 1
                        for h, g, dl_t, off0 in (
                            (0, gA, dlA_t, offA),
                            (1, gB, dlB_t, offB),
                        ):
                            tloc0 = toff[h][b] - off0
                            nt = T[b][h]
                            done = 0
                            while done < nt:
                                k = min(8, nt - done)
                                sel = psel.tile([128, k, 128], bf16, tag="sel")
                                nc.vector.tensor_tensor(
                                    out=sel[:],
                                    in0=dl_t[
                                        :, toff[h][b] + done : toff[h][b] + done + k
                                    ].to_broadcast([128, k, 128]),
                                    in1=iota_t[:, : k * 128],
                                    op=mybir.AluOpType.is_equal,
                                )
                                for q in range(k):
                                    nc.tensor.matmul(
                                        out=acc[:],
                                        lhsT=sel[:, q, :],
                                        rhs=g[
                                            :, tloc0 + done + q,
                                            h * FC : h * FC + FC,
                                        ],
                                        start=False,
                                        stop=(mi == n_mm - 1),
                                    )
                                    mi += 1
                                done += k
                        store(b, acc, scale_t)

            # ---- SpMM1 -> z1 table shard; AllGather
            def store_z1(b, acc, scale_t):
                rows = min(128, NSH - b * 128)
                z1s = pzl1.tile([128, FC], bf16, tag="z1s")
                z1loc[b] = z1s
                nc.scalar.activation(
                    z1s[:], acc[:], COPY, scale=scale_t[:, b : b + 1]
                )
                nc.sync.dma_start(
                    out=z1l_d[b * 128 : b * 128 + rows], in_=z1s[:rows]
                )

            with nc.named_scope("spmm1"):
                spmm(z0f_d[:], z0loc, ivcol_t, store_z1, qoff=0)

            with nc.named_scope("ag1"):
                nc.gpsimd.collective_compute(
                    "AllGather",
                    mybir.AluOpType.bypass,
                    ins=[z1l_d[:]],
                    outs=[z1f_d[:]],
                    replica_groups=[core_ids],
                )

            # ---- SpMM2 -> final output
            def store_out(b, acc, scale_t):
                rows = min(128, NSH - b * 128)
                os_ = pz.tile([128, FC], f32, tag="outs")
                nc.scalar.activation(
                    os_[:], acc[:], COPY, scale=scale_t[:, b : b + 1]
                )
                nc.sync.dma_start(
                    out=out_e[b * 128 : b * 128 + rows], in_=os_[:rows]
                )

            with nc.named_scope("spmm2"):
                spmm(z1f_d[:], z1loc, dvcol_t, store_out, qoff=2)

    nc.compile()
    return nc


_CACHE = {}


def kernel(**inputs):
    in_maps, plan = _prep(**inputs)
    key = tuple(tuple(t) for t in plan["T"])
    if key not in _CACHE:
        _CACHE[key] = _build(plan)
    nc = _CACHE[key]
    res = run_bass_kernel_spmd(nc, in_maps, list(range(NC_CORES)))
    out = np.concatenate(
        [res.results[c]["out"] for c in range(NC_CORES)], axis=0
    )
    if plan["bias_corr"] is not None:
        out = out + plan["bias_corr"]
    mu = np.ascontiguousarray(out[:, :OUT_D])
    lv = np.ascontiguousarray(out[:, OUT_D:])
    return (mu, lv)


# revision 15
# speedup vs baseline: 1.3158x; 1.0834x over previous
"""Two-layer GCN encoder (GCNConv x2 -> mu/logvar heads) on 8 TRN2 NeuronCores.

v3: linear module collapses to [mu | lv] = A^2 @ X @ Wc (Wc = W1 W2 [W_mu|W_lv]).
Folding normalization row-wise (z0 = dinv*(X@Wc), z1 = invdeg*(Ahat z0),
Y = dinv*(Ahat z1)) with Ahat = Adj + I handled as:
  - real edges via dma_gather + one-hot-matmul scatter into PSUM
  - self loops via an identity matmul on the SBUF-resident local z blocks

Device structure (v3):
  - z tables are UNPADDED [N, 64] bf16; the gather views them as pair rows
    [N/2, 128] (256B rows, 2 nodes each). Edge streams are split by src
    parity: even-src edges use gathered cols 0:64, odd-src cols 64:128.
  - AllGather moves 6.4MB tables (halved vs padded layout).
  - Gather sub-calls of <=7 tiles (57 descs/engine packet, under the 64-desc
    HW packet cap) with single_packet=True, rotating 4 SWDGE queues;
    3x descriptor-ring depth (dynamic_dma_scratch_size=49152).
  - scatter-add into PSUM via one-hot matmul (sel as lhsT), per-row scale on
    the scalar engine, local z blocks kept in SBUF for the self-loop term.
"""

import os

import ml_dtypes
import numpy as np

import concourse.bacc as bacc
import concourse.bass as bass
import concourse.mybir as mybir
import concourse.tile as tile
from concourse import library_config
from concourse.bass_utils import run_bass_kernel_spmd

# ---- problem constants (hardcoded per harness contract) ----
N = 50000
IN_D, HID1, HID2, OUT_D = 256, 128, 64, 32
NC_CORES = 8
NSH = N // NC_CORES  # 6250 dst nodes per core
NBLK = (NSH + 127) // 128  # 49 dst blocks per core
NPAIR = N // 2  # pair rows in the gather view (int16-safe: 25000 < 32767)
CHUNK_BLOCKS = 3  # dst blocks per gather chunk
FC = 64  # collapsed feature count
GT = 7  # tiles per gather sub-call: 57 descs/engine packet (<=64 HW cap)

BF16 = ml_dtypes.bfloat16

_tile_patched = False


def _patch_tile_drain():
    """walrus in this env rejects >~2 sem waits on one instruction; Tile's
    kernel-tail drain aggregates one wait per live semaphore. Move the excess
    onto dedicated single-wait SP nops that precede the drain."""
    global _tile_patched
    if _tile_patched:
        return
    _tile_patched = True
    _orig = tile.TileContext._drain_and_barrier

    def _patched(self, tick_clock, wait_clock):
        nc = self.nc
        nops = [nc.sync.nop(nofuse=True, hint=f"dw_{i}").ins for i in range(64)]
        _orig(self, tick_clock, wait_clock)
        ni = 0
        for inst in nc.cur_bb.bb.instructions:
            if "Drain" not in type(inst).__name__:
                continue
            ow = inst.sync_info.on_wait if inst.sync_info else []
            if len(ow) > 1:
                waits = list(ow)
                for w in waits[:-1]:
                    nops[ni].sync_info = mybir.SyncInfo(on_wait=[w], on_update=[])
                    ni += 1
                inst.sync_info.on_wait[:] = waits[-1:]

    tile.TileContext._drain_and_barrier = _patched


def _prep(x, edge_index, W1, b1, W2, b2, W_mu, b_mu, W_lv, b_lv):
    """Host-side graph partitioning + input staging. Returns (in_maps, plan)."""
    src = np.asarray(edge_index[0], dtype=np.int64)
    dst = np.asarray(edge_index[1], dtype=np.int64)

    # degrees include the self loop (handled on-device via identity matmul)
    deg = (np.bincount(dst, minlength=N) + 1).astype(np.float64)
    dinv = deg**-0.5
    invdeg = 1.0 / deg

    # sort real edges by (src-parity, dst): each (dst-block, parity) group
    # contiguous; parity selects gathered cols 0:64 vs 64:128 of a pair row
    par = src % 2
    key = par * N + dst
    order = np.argsort(key, kind="stable")
    s_sorted = src[order]
    d_sorted = dst[order]
    bnd = np.searchsorted(key[order], np.arange(2 * N + 1))

    # per-(core, block, parity) counts -> core-independent tile counts
    T = [[0, 0] for _ in range(NBLK)]
    counts = np.zeros((NC_CORES, NBLK, 2), dtype=np.int64)
    for c in range(NC_CORES):
        for b in range(NBLK):
            lo = c * NSH + b * 128
            hi = min(c * NSH + (b + 1) * 128, (c + 1) * NSH)
            for h in range(2):
                counts[c, b, h] = bnd[h * N + hi] - bnd[h * N + lo]
    MC = [[0, 0] for _ in range(NBLK)]
    for b in range(NBLK):
        for h in range(2):
            MC[b][h] = max(1, int(counts[:, b, h].max()))
            T[b][h] = -(-MC[b][h] // 128)

    TH = [sum(T[b][h] for b in range(NBLK)) for h in range(2)]
    toff = [[0] * NBLK, [0] * NBLK]
    for h in range(2):
        acc = 0
        for b in range(NBLK):
            toff[h][b] = acc
            acc += T[b][h]

    # per-core padded idx / dstloc streams (idx = pair row = src // 2)
    core_data = []
    for c in range(NC_CORES):
        idx_streams = []
        dl_streams = []
        for h in range(2):
            idx = np.zeros(TH[h] * 128, dtype=np.int16)
            dl = np.full(TH[h] * 128, -1.0, dtype=np.float32)
            for b in range(NBLK):
                lo = c * NSH + b * 128
                hi = min(c * NSH + (b + 1) * 128, (c + 1) * NSH)
                e0, e1 = bnd[h * N + lo], bnd[h * N + hi]
                cnt = e1 - e0
                off = toff[h][b] * 128
                idx[off : off + cnt] = (s_sorted[e0:e1] // 2).astype(np.int16)
                dl[off : off + cnt] = (d_sorted[e0:e1] - lo).astype(np.float32)
            packed = np.tile(np.ascontiguousarray(idx.reshape(-1, 16).T), (8, 1))
            idx_streams.append(packed)
            dl_streams.append(np.ascontiguousarray(dl.reshape(-1, 128).T).astype(BF16))
        core_data.append((idx_streams, dl_streams))

    # collapsed weights
    W1_ = np.asarray(W1, np.float64)
    W2_ = np.asarray(W2, np.float64)
    Wh = np.concatenate(
        [np.asarray(W_mu, np.float64), np.asarray(W_lv, np.float64)], axis=1
    )  # [64, 64]
    Wc = W1_ @ W2_ @ Wh  # [256, 64]
    wca = Wc[:128].astype(BF16)
    wcb = Wc[128:].astype(BF16)

    # host-side bias correction (zero for this module)
    r1 = (np.asarray(b1, np.float64) @ W2_) @ Wh  # [64]
    r0 = np.asarray(b2, np.float64) @ Wh + np.concatenate(
        [np.asarray(b_mu, np.float64), np.asarray(b_lv, np.float64)]
    )
    if np.any(r1) or np.any(r0):
        s_vec = dinv * (
            np.bincount(dst, weights=dinv[src], minlength=N) + dinv
        )
        bias_corr = (s_vec[:, None] * r1[None, :] + r0[None, :]).astype(np.float32)
    else:
        bias_corr = None

    iota_rep = np.tile(np.arange(128, dtype=np.float32), (128, 8)).astype(BF16)
    ident = np.eye(128, dtype=np.float32).astype(BF16)

    xf = np.asarray(x, np.float32)
    in_maps = []
    for c in range(NC_CORES):
        (idxA, idxB), (dlA, dlB) = core_data[c]
        own = slice(c * NSH, (c + 1) * NSH)
        xsh = np.zeros((IN_D, NBLK * 128), np.float32)
        xsh[:, :NSH] = xf[own].T
        tmp_iv = np.zeros(NBLK * 128, np.float64)
        tmp_dv = np.zeros(NBLK * 128, np.float64)
        tmp_iv[:NSH] = invdeg[own]
        tmp_dv[:NSH] = dinv[own]
        in_maps.append(
            {
                "xsh": xsh.astype(BF16),
                "iota": iota_rep,
                "ident": ident,
                "idxA": idxA,
                "idxB": idxB,
                "dlA": dlA,
                "dlB": dlB,
                "wca": wca,
                "wcb": wcb,
                "ivcol": np.ascontiguousarray(
                    tmp_iv.reshape(NBLK, 128).T
                ).astype(np.float32),
                "dvcol": np.ascontiguousarray(
                    tmp_dv.reshape(NBLK, 128).T
                ).astype(np.float32),
            }
        )

    plan = {"T": T, "TH": TH, "toff": toff, "MC": MC, "bias_corr": bias_corr}
    return in_maps, plan


def _build(plan):
    _patch_tile_drain()
    T, TH, toff, MC = plan["T"], plan["TH"], plan["toff"], plan["MC"]

    nc = bacc.Bacc("TRN2", num_swdge_queues=4, dynamic_dma_scratch_size=49152)
    f32, bf16, i16 = mybir.dt.float32, mybir.dt.bfloat16, mybir.dt.int16
    COPY = mybir.ActivationFunctionType.Copy

    xsh_e = nc.dram_tensor("xsh", [IN_D, NBLK * 128], bf16, kind="ExternalInput")
    iota_e = nc.dram_tensor("iota", [128, 1024], bf16, kind="ExternalInput")
    ident_e = nc.dram_tensor("ident", [128, 128], bf16, kind="ExternalInput")
    idxA_e = nc.dram_tensor("idxA", [128, TH[0] * 8], i16, kind="ExternalInput")
    idxB_e = nc.dram_tensor("idxB", [128, TH[1] * 8], i16, kind="ExternalInput")
    dlA_e = nc.dram_tensor("dlA", [128, TH[0]], bf16, kind="ExternalInput")
    dlB_e = nc.dram_tensor("dlB", [128, TH[1]], bf16, kind="ExternalInput")
    wca_e = nc.dram_tensor("wca", [128, FC], bf16, kind="ExternalInput")
    wcb_e = nc.dram_tensor("wcb", [128, FC], bf16, kind="ExternalInput")
    ivcol_e = nc.dram_tensor("ivcol", [128, NBLK], f32, kind="ExternalInput")
    dvcol_e = nc.dram_tensor("dvcol", [128, NBLK], f32, kind="ExternalInput")

    out_e = nc.dram_tensor("out", [NSH, FC], f32, kind="ExternalOutput")

    z0l_d = nc.dram_tensor("z0l_d", [NSH, FC], bf16)
    z0f_d = nc.dram_tensor("z0f_d", [NPAIR, 2 * FC], bf16, addr_space="Shared")
    z1l_d = nc.dram_tensor("z1l_d", [NSH, FC], bf16)
    z1f_d = nc.dram_tensor("z1f_d", [NPAIR, 2 * FC], bf16, addr_space="Shared")

    core_ids = list(range(NC_CORES))

    chunks = []
    b0 = 0
    while b0 < NBLK:
        chunks.append((b0, min(b0 + CHUNK_BLOCKS, NBLK)))
        b0 = min(b0 + CHUNK_BLOCKS, NBLK)

    with tile.TileContext(nc) as tc:
        with (
            tc.tile_pool(name="const", bufs=1) as pc,
            tc.tile_pool(name="xa", bufs=3) as px,
            tc.tile_pool(name="zl0", bufs=NBLK) as pzl0,
            tc.tile_pool(name="zl1", bufs=NBLK) as pzl1,
            tc.tile_pool(name="zb", bufs=4) as pz,
            tc.tile_pool(name="g", bufs=6) as pg,
            tc.tile_pool(name="sel", bufs=8) as psel,
            tc.tile_pool(name="psA", bufs=2, space="PSUM") as ppA,
            tc.tile_pool(name="psacc", bufs=6, space="PSUM") as ppa,
        ):
            nc.gpsimd.load_library(library_config.mlp)

            # ---- resident constants
            iota_t = pc.tile([128, 1024], bf16)
            nc.sync.dma_start(out=iota_t[:], in_=iota_e[:])
            ident_t = pc.tile([128, 128], bf16)
            nc.sync.dma_start(out=ident_t[:], in_=ident_e[:])
            idxA_t = pc.tile([128, TH[0] * 8], i16)
            nc.sync.dma_start(out=idxA_t[:], in_=idxA_e[:])
            idxB_t = pc.tile([128, TH[1] * 8], i16)
            nc.sync.dma_start(out=idxB_t[:], in_=idxB_e[:])
            dlA_t = pc.tile([128, TH[0]], bf16)
            nc.sync.dma_start(out=dlA_t[:], in_=dlA_e[:])
            dlB_t = pc.tile([128, TH[1]], bf16)
            nc.sync.dma_start(out=dlB_t[:], in_=dlB_e[:])
            wca_t = pc.tile([128, FC], bf16)
            nc.sync.dma_start(out=wca_t[:], in_=wca_e[:])
            wcb_t = pc.tile([128, FC], bf16)
            nc.sync.dma_start(out=wcb_t[:], in_=wcb_e[:])
            ivcol_t = pc.tile([128, NBLK], f32)
            nc.sync.dma_start(out=ivcol_t[:], in_=ivcol_e[:])
            dvcol_t = pc.tile([128, NBLK], f32)
            nc.sync.dma_start(out=dvcol_t[:], in_=dvcol_e[:])

            z0loc = [None] * NBLK
            z1loc = [None] * NBLK

            # ---- phase A: z0 shard = dinv * (x_shard @ Wc)
            with nc.named_scope("phaseA"):
                done = 0
                while done < NBLK:
                    nb_cnt = min(8, NBLK - done)
                    c0 = done * 128
                    cols = nb_cnt * 128
                    xa = px.tile([128, 1024], bf16, tag="xa")
                    xb = px.tile([128, 1024], bf16, tag="xb")
                    nc.sync.dma_start(
                        out=xa[:, :cols], in_=xsh_e[0:128, c0 : c0 + cols]
                    )
                    nc.scalar.dma_start(
                        out=xb[:, :cols], in_=xsh_e[128:256, c0 : c0 + cols]
                    )
                    for j in range(nb_cnt):
                        gb = done + j
                        rows = min(128, NSH - gb * 128)
                        zp = ppA.tile([128, FC], f32, space="PSUM", tag="zp")
                        nc.tensor.matmul(
                            out=zp[:],
                            lhsT=xa[:, j * 128 : (j + 1) * 128],
                            rhs=wca_t[:],
                            start=True,
                            stop=False,
                        )
                        nc.tensor.matmul(
                            out=zp[:],
                            lhsT=xb[:, j * 128 : (j + 1) * 128],
                            rhs=wcb_t[:],
                            start=False,
                            stop=True,
                        )
                        z0s = pzl0.tile([128, FC], bf16, tag="z0s")
                        z0loc[gb] = z0s
                        nc.scalar.activation(
                            z0s[:], zp[:], COPY,
                            scale=dvcol_t[:, gb : gb + 1],
                        )
                        nc.sync.dma_start(
                            out=z0l_d[gb * 128 : gb * 128 + rows], in_=z0s[:rows]
                        )
                    done += nb_cnt

            with nc.named_scope("ag0"):
                nc.gpsimd.collective_compute(
                    "AllGather",
                    mybir.AluOpType.bypass,
                    ins=[z0l_d[:]],
                    outs=[z0f_d[:]],
                    replica_groups=[core_ids],
                )

            # ---- shared SpMM: acc[dst_block] = z_self[block] + sum_e msg[e]
            def spmm(src_d, zloc, scale_t, store, qoff=0, init_ms=False):
                qn = qoff
                for ci, (cb0, cb1) in enumerate(chunks):
                    ctA = sum(T[b][0] for b in range(cb0, cb1))
                    ctB = sum(T[b][1] for b in range(cb0, cb1))
                    offA, offB = toff[0][cb0], toff[1][cb0]
                    gA = pg.tile([128, ctA, 128], bf16, tag="gA")
                    gB = pg.tile([128, ctB, 128], bf16, tag="gB")
                    if init_ms and ci < 6:
                        # first rotation of the pool: clear stale SBUF so
                        # un-gathered pad slots can't feed NaNs to the PE
                        nc.vector.memset(gA[:], 0.0)
                        nc.vector.memset(gB[:], 0.0)
                    for h, g, idx_t, off in (
                        (0, gA, idxA_t, offA),
                        (1, gB, idxB_t, offB),
                    ):
                        for b in range(cb0, cb1):
                            tb0 = toff[h][b] - off
                            nt = T[b][h]
                            mc = MC[b][h]
                            t0 = 0
                            while t0 < nt:
                                tn = min(GT, nt - t0)
                                ni = max(1, min(tn * 128, mc - t0 * 128))
                                nc.gpsimd.dma_gather(
                                    g[:, tb0 + t0 : tb0 + t0 + tn, :], src_d,
                                    idx_t[
                                        :,
                                        (toff[h][b] + t0) * 8 :
                                        (toff[h][b] + t0 + tn) * 8,
                                    ],
                                    ni, ni, 128,
                                    single_packet=True, queue_num=qn % 4,
                                )
                                qn += 1
                                t0 += tn
                    for b in range(cb0, cb1):
                        acc = ppa.tile([128, FC], f32, space="PSUM", tag="acc")
                        n_mm = 1 + T[b][0] + T[b][1]
                        nc.tensor.matmul(
                            out=acc[:],
                            lhsT=ident_t[:],
                            rhs=zloc[b][:],
                            start=True,
                            stop=False,
                        )
                        mi = 1
                        for h, g, dl_t, off0 in (
                            (0, gA, dlA_t, offA),
                            (1, gB, dlB_t, offB),
                        ):
                            tloc0 = toff[h][b] - off0
                            nt = T[b][h]
                            done = 0
                            while done < nt:
                                k = min(8, nt - done)
                                sel = psel.tile([128, k, 128], bf16, tag="sel")
                                nc.vector.tensor_tensor(
                                    out=sel[:],
                                    in0=dl_t[
                                        :, toff[h][b] + done : toff[h][b] + done + k
                                    ].to_broadcast([128, k, 128]),
                                    in1=iota_t[:, : k * 128],
                                    op=mybir.AluOpType.is_equal,
                                )
                                for q in range(k):
                                    nc.tensor.matmul(
                                        out=acc[:],
                                        lhsT=sel[:, q, :],
                                        rhs=g[
                                            :, tloc0 + done + q,
                                            h * FC : h * FC + FC,
                                        ],
                                        start=False,
                                        stop=(mi == n_mm - 1),
                                    )
                                    mi += 1
                                done += k
                        store(b, acc, scale_t)

            # ---- SpMM1 -> z1 table shard; AllGather
            def store_z1(b, acc, scale_t):
                rows = min(128, NSH - b * 128)
                z1s = pzl1.tile([128, FC], bf16, tag="z1s")
                z1loc[b] = z1s
                nc.scalar.activation(
                    z1s[:], acc[:], COPY, scale=scale_t[:, b : b + 1]
                )
                nc.sync.dma_start(
                    out=z1l_d[b * 128 : b * 128 + rows], in_=z1s[:rows]
                )

            with nc.named_scope("spmm1"):
                spmm(z0f_d[:], z0loc, ivcol_t, store_z1, qoff=0, init_ms=True)

            with nc.named_scope("ag1"):
                nc.gpsimd.collective_compute(
                    "AllGather",
                    mybir.AluOpType.bypass,
                    ins=[z1l_d[:]],
                    outs=[z1f_d[:]],
                    replica_groups=[core_ids],
                )

            # ---- SpMM2 -> final output
            def store_out(b, acc, scale_t):
                rows = min(128, NSH - b * 128)
                os_ = pz.tile([128, FC], f32, tag="outs")
                nc.scalar.activation(
                    os_[:], acc[:], COPY, scale=scale_t[:, b : b + 1]
                )
                nc.sync.dma_start(
                    out=out_e[b * 128 : b * 128 + rows], in_=os_[:rows]
                )

            with nc.named_scope("spmm2"):
                spmm(z1f_d[:], z1loc, dvcol_t, store_out, qoff=2)

    nc.compile()
    return nc


_CACHE = {}


def kernel(**inputs):
    in_maps, plan = _prep(**inputs)
    key = tuple(tuple(t) for t in plan["MC"])
    if key not in _CACHE:
        _CACHE[key] = _build(plan)
    nc = _CACHE[key]
    res = run_bass_kernel_spmd(nc, in_maps, list(range(NC_CORES)))
    out = np.concatenate(
        [res.results[c]["out"] for c in range(NC_CORES)], axis=0
    )
    if plan["bias_corr"] is not None:
        out = out + plan["bias_corr"]
    mu = np.ascontiguousarray(out[:, :OUT_D])
    lv = np.ascontiguousarray(out[:, OUT_D:])
    return (mu, lv)


# revision 26
# speedup vs baseline: 1.3200x; 1.0032x over previous
"""Two-layer GCN encoder (GCNConv x2 -> mu/logvar heads) on 8 TRN2 NeuronCores.

v3: linear module collapses to [mu | lv] = A^2 @ X @ Wc (Wc = W1 W2 [W_mu|W_lv]).
Folding normalization row-wise (z0 = dinv*(X@Wc), z1 = invdeg*(Ahat z0),
Y = dinv*(Ahat z1)) with Ahat = Adj + I handled as:
  - real edges via dma_gather + one-hot-matmul scatter into PSUM
  - self loops via an identity matmul on the SBUF-resident local z blocks

Device structure (v3):
  - z tables are UNPADDED [N, 64] bf16; the gather views them as pair rows
    [N/2, 128] (256B rows, 2 nodes each). Edge streams are split by src
    parity: even-src edges use gathered cols 0:64, odd-src cols 64:128.
  - AllGather moves 6.4MB tables (halved vs padded layout).
  - Gather sub-calls of <=7 tiles (57 descs/engine packet, under the 64-desc
    HW packet cap) with single_packet=True, rotating 4 SWDGE queues;
    3x descriptor-ring depth (dynamic_dma_scratch_size=49152).
  - scatter-add into PSUM via one-hot matmul (sel as lhsT), per-row scale on
    the scalar engine, local z blocks kept in SBUF for the self-loop term.
"""

import os

import ml_dtypes
import numpy as np

import concourse.bacc as bacc
import concourse.bass as bass
import concourse.mybir as mybir
import concourse.tile as tile
from concourse import library_config
from concourse.bass_utils import run_bass_kernel_spmd

# ---- problem constants (hardcoded per harness contract) ----
N = 50000
IN_D, HID1, HID2, OUT_D = 256, 128, 64, 32
NC_CORES = 8
NSH = N // NC_CORES  # 6250 dst nodes per core
NBLK = (NSH + 127) // 128  # 49 dst blocks per core
NPAIR = N // 2  # pair rows in the gather view (int16-safe: 25000 < 32767)
CHUNK_BLOCKS = 3  # dst blocks per gather chunk
FC = 64  # collapsed feature count
GT = 7  # tiles per gather sub-call: 57 descs/engine packet (<=64 HW cap)

BF16 = ml_dtypes.bfloat16

_tile_patched = False


def _patch_tile_drain():
    """walrus in this env rejects >~2 sem waits on one instruction; Tile's
    kernel-tail drain aggregates one wait per live semaphore. Move the excess
    onto dedicated single-wait SP nops that precede the drain."""
    global _tile_patched
    if _tile_patched:
        return
    _tile_patched = True
    _orig = tile.TileContext._drain_and_barrier

    def _patched(self, tick_clock, wait_clock):
        nc = self.nc
        nops = [nc.sync.nop(nofuse=True, hint=f"dw_{i}").ins for i in range(64)]
        _orig(self, tick_clock, wait_clock)
        ni = 0
        for inst in nc.cur_bb.bb.instructions:
            if "Drain" not in type(inst).__name__:
                continue
            ow = inst.sync_info.on_wait if inst.sync_info else []
            if len(ow) > 1:
                waits = list(ow)
                for w in waits[:-1]:
                    nops[ni].sync_info = mybir.SyncInfo(on_wait=[w], on_update=[])
                    ni += 1
                inst.sync_info.on_wait[:] = waits[-1:]

    tile.TileContext._drain_and_barrier = _patched


def _prep(x, edge_index, W1, b1, W2, b2, W_mu, b_mu, W_lv, b_lv):
    """Host-side graph partitioning + input staging. Returns (in_maps, plan)."""
    src = np.asarray(edge_index[0], dtype=np.int64)
    dst = np.asarray(edge_index[1], dtype=np.int64)

    # degrees include the self loop (handled on-device via identity matmul)
    deg = (np.bincount(dst, minlength=N) + 1).astype(np.float64)
    dinv = deg**-0.5
    invdeg = 1.0 / deg

    # sort real edges by (src-parity, dst): each (dst-block, parity) group
    # contiguous; parity selects gathered cols 0:64 vs 64:128 of a pair row
    par = src % 2
    key = par * N + dst
    order = np.argsort(key, kind="stable")
    s_sorted = src[order]
    d_sorted = dst[order]
    bnd = np.searchsorted(key[order], np.arange(2 * N + 1))

    # per-(core, block, parity) counts -> core-independent tile counts
    T = [[0, 0] for _ in range(NBLK)]
    counts = np.zeros((NC_CORES, NBLK, 2), dtype=np.int64)
    for c in range(NC_CORES):
        for b in range(NBLK):
            lo = c * NSH + b * 128
            hi = min(c * NSH + (b + 1) * 128, (c + 1) * NSH)
            for h in range(2):
                counts[c, b, h] = bnd[h * N + hi] - bnd[h * N + lo]
    MC = [[0, 0] for _ in range(NBLK)]
    for b in range(NBLK):
        for h in range(2):
            MC[b][h] = max(1, int(counts[:, b, h].max()))
            T[b][h] = -(-MC[b][h] // 128)

    TH = [sum(T[b][h] for b in range(NBLK)) for h in range(2)]
    toff = [[0] * NBLK, [0] * NBLK]
    for h in range(2):
        acc = 0
        for b in range(NBLK):
            toff[h][b] = acc
            acc += T[b][h]

    # per-core padded idx / dstloc streams (idx = pair row = src // 2)
    core_data = []
    for c in range(NC_CORES):
        idx_streams = []
        dl_streams = []
        for h in range(2):
            idx = np.zeros(TH[h] * 128, dtype=np.int16)
            dl = np.full(TH[h] * 128, -1.0, dtype=np.float32)
            for b in range(NBLK):
                lo = c * NSH + b * 128
                hi = min(c * NSH + (b + 1) * 128, (c + 1) * NSH)
                e0, e1 = bnd[h * N + lo], bnd[h * N + hi]
                cnt = e1 - e0
                off = toff[h][b] * 128
                idx[off : off + cnt] = (s_sorted[e0:e1] // 2).astype(np.int16)
                dl[off : off + cnt] = (d_sorted[e0:e1] - lo).astype(np.float32)
            packed = np.tile(np.ascontiguousarray(idx.reshape(-1, 16).T), (8, 1))
            idx_streams.append(packed)
            dl_streams.append(np.ascontiguousarray(dl.reshape(-1, 128).T).astype(BF16))
        core_data.append((idx_streams, dl_streams))

    # collapsed weights
    W1_ = np.asarray(W1, np.float64)
    W2_ = np.asarray(W2, np.float64)
    Wh = np.concatenate(
        [np.asarray(W_mu, np.float64), np.asarray(W_lv, np.float64)], axis=1
    )  # [64, 64]
    Wc = W1_ @ W2_ @ Wh  # [256, 64]
    wca = Wc[:128].astype(BF16)
    wcb = Wc[128:].astype(BF16)

    # host-side bias correction (zero for this module)
    r1 = (np.asarray(b1, np.float64) @ W2_) @ Wh  # [64]
    r0 = np.asarray(b2, np.float64) @ Wh + np.concatenate(
        [np.asarray(b_mu, np.float64), np.asarray(b_lv, np.float64)]
    )
    if np.any(r1) or np.any(r0):
        s_vec = dinv * (
            np.bincount(dst, weights=dinv[src], minlength=N) + dinv
        )
        bias_corr = (s_vec[:, None] * r1[None, :] + r0[None, :]).astype(np.float32)
    else:
        bias_corr = None

    iota_rep = np.tile(np.arange(128, dtype=np.float32), (128, 8)).astype(BF16)
    ident = np.eye(128, dtype=np.float32).astype(BF16)

    xf = np.asarray(x, np.float32)
    in_maps = []
    for c in range(NC_CORES):
        (idxA, idxB), (dlA, dlB) = core_data[c]
        own = slice(c * NSH, (c + 1) * NSH)
        xsh = np.zeros((IN_D, NBLK * 128), np.float32)
        xsh[:, :NSH] = xf[own].T
        tmp_iv = np.zeros(NBLK * 128, np.float64)
        tmp_dv = np.zeros(NBLK * 128, np.float64)
        tmp_iv[:NSH] = invdeg[own]
        tmp_dv[:NSH] = dinv[own]
        in_maps.append(
            {
                "xsh": xsh.astype(BF16),
                "iota": iota_rep,
                "ident": ident,
                "idxA": idxA,
                "idxB": idxB,
                "dlA": dlA,
                "dlB": dlB,
                "wca": wca,
                "wcb": wcb,
                "ivcol": np.ascontiguousarray(
                    tmp_iv.reshape(NBLK, 128).T
                ).astype(np.float32),
                "dvcol": np.ascontiguousarray(
                    tmp_dv.reshape(NBLK, 128).T
                ).astype(np.float32),
            }
        )

    plan = {"T": T, "TH": TH, "toff": toff, "MC": MC, "bias_corr": bias_corr}
    return in_maps, plan


def _build(plan):
    _patch_tile_drain()
    T, TH, toff, MC = plan["T"], plan["TH"], plan["toff"], plan["MC"]

    nc = bacc.Bacc("TRN2", num_swdge_queues=4, dynamic_dma_scratch_size=49152)
    f32, bf16, i16 = mybir.dt.float32, mybir.dt.bfloat16, mybir.dt.int16
    COPY = mybir.ActivationFunctionType.Copy

    xsh_e = nc.dram_tensor("xsh", [IN_D, NBLK * 128], bf16, kind="ExternalInput")
    iota_e = nc.dram_tensor("iota", [128, 1024], bf16, kind="ExternalInput")
    ident_e = nc.dram_tensor("ident", [128, 128], bf16, kind="ExternalInput")
    idxA_e = nc.dram_tensor("idxA", [128, TH[0] * 8], i16, kind="ExternalInput")
    idxB_e = nc.dram_tensor("idxB", [128, TH[1] * 8], i16, kind="ExternalInput")
    dlA_e = nc.dram_tensor("dlA", [128, TH[0]], bf16, kind="ExternalInput")
    dlB_e = nc.dram_tensor("dlB", [128, TH[1]], bf16, kind="ExternalInput")
    wca_e = nc.dram_tensor("wca", [128, FC], bf16, kind="ExternalInput")
    wcb_e = nc.dram_tensor("wcb", [128, FC], bf16, kind="ExternalInput")
    ivcol_e = nc.dram_tensor("ivcol", [128, NBLK], f32, kind="ExternalInput")
    dvcol_e = nc.dram_tensor("dvcol", [128, NBLK], f32, kind="ExternalInput")

    out_e = nc.dram_tensor("out", [NSH, FC], f32, kind="ExternalOutput")

    z0l_d = nc.dram_tensor("z0l_d", [NSH, FC], bf16)
    z0f_d = nc.dram_tensor("z0f_d", [NPAIR, 2 * FC], bf16, addr_space="Shared")
    z1l_d = nc.dram_tensor("z1l_d", [NSH, FC], bf16)
    z1f_d = nc.dram_tensor("z1f_d", [NPAIR, 2 * FC], bf16, addr_space="Shared")

    core_ids = list(range(NC_CORES))

    chunks = []
    b0 = 0
    while b0 < NBLK:
        chunks.append((b0, min(b0 + CHUNK_BLOCKS, NBLK)))
        b0 = min(b0 + CHUNK_BLOCKS, NBLK)

    with tile.TileContext(nc) as tc:
        with (
            tc.tile_pool(name="const", bufs=1) as pc,
            tc.tile_pool(name="xa", bufs=3) as px,
            tc.tile_pool(name="zl0", bufs=NBLK) as pzl0,
            tc.tile_pool(name="zl1", bufs=NBLK) as pzl1,
            tc.tile_pool(name="zb", bufs=4) as pz,
            tc.tile_pool(name="g", bufs=6) as pg,
            tc.tile_pool(name="sel", bufs=8) as psel,
            tc.tile_pool(name="psA", bufs=2, space="PSUM") as ppA,
            tc.tile_pool(name="psacc", bufs=6, space="PSUM") as ppa,
        ):
            nc.gpsimd.load_library(library_config.mlp)

            # ---- resident constants
            iota_t = pc.tile([128, 1024], bf16)
            nc.sync.dma_start(out=iota_t[:], in_=iota_e[:])
            ident_t = pc.tile([128, 128], bf16)
            nc.sync.dma_start(out=ident_t[:], in_=ident_e[:])
            idxA_t = pc.tile([128, TH[0] * 8], i16)
            nc.sync.dma_start(out=idxA_t[:], in_=idxA_e[:])
            idxB_t = pc.tile([128, TH[1] * 8], i16)
            nc.sync.dma_start(out=idxB_t[:], in_=idxB_e[:])
            dlA_t = pc.tile([128, TH[0]], bf16)
            nc.sync.dma_start(out=dlA_t[:], in_=dlA_e[:])
            dlB_t = pc.tile([128, TH[1]], bf16)
            nc.sync.dma_start(out=dlB_t[:], in_=dlB_e[:])
            wca_t = pc.tile([128, FC], bf16)
            nc.sync.dma_start(out=wca_t[:], in_=wca_e[:])
            wcb_t = pc.tile([128, FC], bf16)
            nc.sync.dma_start(out=wcb_t[:], in_=wcb_e[:])
            ivcol_t = pc.tile([128, NBLK], f32)
            nc.sync.dma_start(out=ivcol_t[:], in_=ivcol_e[:])
            dvcol_t = pc.tile([128, NBLK], f32)
            nc.sync.dma_start(out=dvcol_t[:], in_=dvcol_e[:])

            z0loc = [None] * NBLK
            z1loc = [None] * NBLK

            # ---- phase A: z0 shard = dinv * (x_shard @ Wc)
            with nc.named_scope("phaseA"):
                done = 0
                while done < NBLK:
                    nb_cnt = min(8, NBLK - done)
                    c0 = done * 128
                    cols = nb_cnt * 128
                    xa = px.tile([128, 1024], bf16, tag="xa")
                    xb = px.tile([128, 1024], bf16, tag="xb")
                    nc.sync.dma_start(
                        out=xa[:, :cols], in_=xsh_e[0:128, c0 : c0 + cols]
                    )
                    nc.scalar.dma_start(
                        out=xb[:, :cols], in_=xsh_e[128:256, c0 : c0 + cols]
                    )
                    for j in range(nb_cnt):
                        gb = done + j
                        rows = min(128, NSH - gb * 128)
                        zp = ppA.tile([128, FC], f32, space="PSUM", tag="zp")
                        nc.tensor.matmul(
                            out=zp[:],
                            lhsT=xa[:, j * 128 : (j + 1) * 128],
                            rhs=wca_t[:],
                            start=True,
                            stop=False,
                        )
                        nc.tensor.matmul(
                            out=zp[:],
                            lhsT=xb[:, j * 128 : (j + 1) * 128],
                            rhs=wcb_t[:],
                            start=False,
                            stop=True,
                        )
                        z0s = pzl0.tile([128, FC], bf16, tag="z0s")
                        z0loc[gb] = z0s
                        nc.scalar.activation(
                            z0s[:], zp[:], COPY,
                            scale=dvcol_t[:, gb : gb + 1],
                        )
                        nc.sync.dma_start(
                            out=z0l_d[gb * 128 : gb * 128 + rows], in_=z0s[:rows]
                        )
                    done += nb_cnt

            with nc.named_scope("ag0"):
                nc.gpsimd.collective_compute(
                    "AllGather",
                    mybir.AluOpType.bypass,
                    ins=[z0l_d[:]],
                    outs=[z0f_d[:]],
                    replica_groups=[core_ids],
                )

            # ---- shared SpMM: acc[dst_block] = z_self[block] + sum_e msg[e]
            def spmm(src_d, zloc, scale_t, store, qoff=0, init_ms=False):
                qn = qoff
                for ci, (cb0, cb1) in enumerate(chunks):
                    ctA = sum(T[b][0] for b in range(cb0, cb1))
                    ctB = sum(T[b][1] for b in range(cb0, cb1))
                    offA, offB = toff[0][cb0], toff[1][cb0]
                    gA = pg.tile([128, ctA, 128], bf16, tag="gA")
                    gB = pg.tile([128, ctB, 128], bf16, tag="gB")
                    if init_ms and ci < 6:
                        # first rotation of the pool: clear stale SBUF so
                        # un-gathered pad slots can't feed NaNs to the PE
                        nc.vector.memset(gA[:], 0.0)
                        nc.vector.memset(gB[:], 0.0)
                    for h, g, idx_t, off in (
                        (0, gA, idxA_t, offA),
                        (1, gB, idxB_t, offB),
                    ):
                        for b in range(cb0, cb1):
                            tb0 = toff[h][b] - off
                            nt = T[b][h]
                            mc = MC[b][h]
                            t0 = 0
                            while t0 < nt:
                                tn = min(GT, nt - t0)
                                ni = max(1, min(tn * 128, mc - t0 * 128))
                                nc.gpsimd.dma_gather(
                                    g[:, tb0 + t0 : tb0 + t0 + tn, :], src_d,
                                    idx_t[
                                        :,
                                        (toff[h][b] + t0) * 8 :
                                        (toff[h][b] + t0 + tn) * 8,
                                    ],
                                    ni, ni, 128,
                                    single_packet=True, queue_num=qn % 4,
                                )
                                qn += 1
                                t0 += tn
                    for b in range(cb0, cb1):
                        acc = ppa.tile([128, FC], f32, space="PSUM", tag="acc")
                        n_mm = 1 + T[b][0] + T[b][1]
                        nc.tensor.matmul(
                            out=acc[:],
                            lhsT=ident_t[:],
                            rhs=zloc[b][:],
                            start=True,
                            stop=False,
                        )
                        mi = 1
                        for h, g, dl_t, off0 in (
                            (0, gA, dlA_t, offA),
                            (1, gB, dlB_t, offB),
                        ):
                            tloc0 = toff[h][b] - off0
                            nt = T[b][h]
                            done = 0
                            while done < nt:
                                k = min(8, nt - done)
                                sel = psel.tile([128, k, 128], bf16, tag="sel")
                                nc.vector.tensor_tensor(
                                    out=sel[:],
                                    in0=dl_t[
                                        :, toff[h][b] + done : toff[h][b] + done + k
                                    ].to_broadcast([128, k, 128]),
                                    in1=iota_t[:, : k * 128],
                                    op=mybir.AluOpType.is_equal,
                                )
                                for q in range(k):
                                    nc.tensor.matmul(
                                        out=acc[:],
                                        lhsT=sel[:, q, :],
                                        rhs=g[
                                            :, tloc0 + done + q,
                                            h * FC : h * FC + FC,
                                        ],
                                        start=False,
                                        stop=(mi == n_mm - 1),
                                    )
                                    mi += 1
                                done += k
                        store(b, acc, scale_t)

            # ---- SpMM1 -> z1 table shard; AllGather
            def store_z1(b, acc, scale_t):
                rows = min(128, NSH - b * 128)
                z1s = pzl1.tile([128, FC], bf16, tag="z1s")
                z1loc[b] = z1s
                nc.scalar.activation(
                    z1s[:], acc[:], COPY, scale=scale_t[:, b : b + 1]
                )
                nc.sync.dma_start(
                    out=z1l_d[b * 128 : b * 128 + rows], in_=z1s[:rows]
                )

            with nc.named_scope("spmm1"):
                spmm(z0f_d[:], z0loc, ivcol_t, store_z1, qoff=0, init_ms=True)

            with nc.named_scope("ag1"):
                nc.gpsimd.collective_compute(
                    "AllGather",
                    mybir.AluOpType.bypass,
                    ins=[z1l_d[:]],
                    outs=[z1f_d[:]],
                    replica_groups=[core_ids],
                )

            # ---- SpMM2 -> final output
            def store_out(b, acc, scale_t):
                rows = min(128, NSH - b * 128)
                os_ = pz.tile([128, FC], f32, tag="outs")
                nc.scalar.activation(
                    os_[:], acc[:], COPY, scale=scale_t[:, b : b + 1]
                )
                nc.sync.dma_start(
                    out=out_e[b * 128 : b * 128 + rows], in_=os_[:rows]
                )

            with nc.named_scope("spmm2"):
                spmm(z1f_d[:], z1loc, dvcol_t, store_out, qoff=2)

    nc.compile()
    return nc


_CACHE = {}


def kernel(**inputs):
    in_maps, plan = _prep(**inputs)
    key = tuple(tuple(t) for t in plan["MC"])
    if key not in _CACHE:
        _CACHE[key] = _build(plan)
    nc = _CACHE[key]
    res = run_bass_kernel_spmd(nc, in_maps, list(range(NC_CORES)))
    out = np.concatenate(
        [res.results[c]["out"] for c in range(NC_CORES)], axis=0
    )
    if plan["bias_corr"] is not None:
        out = out + plan["bias_corr"]
    mu = np.ascontiguousarray(out[:, :OUT_D])
    lv = np.ascontiguousarray(out[:, OUT_D:])
    return (mu, lv)


# revision 30
# speedup vs baseline: 1.3232x; 1.0024x over previous
"""Two-layer GCN encoder (GCNConv x2 -> mu/logvar heads) on 8 TRN2 NeuronCores.

v3: linear module collapses to [mu | lv] = A^2 @ X @ Wc (Wc = W1 W2 [W_mu|W_lv]).
Folding normalization row-wise (z0 = dinv*(X@Wc), z1 = invdeg*(Ahat z0),
Y = dinv*(Ahat z1)) with Ahat = Adj + I handled as:
  - real edges via dma_gather + one-hot-matmul scatter into PSUM
  - self loops via an identity matmul on the SBUF-resident local z blocks

Device structure (v3):
  - z tables are UNPADDED [N, 64] bf16; the gather views them as pair rows
    [N/2, 128] (256B rows, 2 nodes each). Edge streams are split by src
    parity: even-src edges use gathered cols 0:64, odd-src cols 64:128.
  - AllGather moves 6.4MB tables (halved vs padded layout).
  - Gather sub-calls of <=7 tiles (57 descs/engine packet, under the 64-desc
    HW packet cap) with single_packet=True, rotating 4 SWDGE queues;
    3x descriptor-ring depth (dynamic_dma_scratch_size=49152).
  - scatter-add into PSUM via one-hot matmul (sel as lhsT), per-row scale on
    the scalar engine, local z blocks kept in SBUF for the self-loop term.
"""

import os

import ml_dtypes
import numpy as np

import concourse.bacc as bacc
import concourse.bass as bass
import concourse.mybir as mybir
import concourse.tile as tile
from concourse import library_config
from concourse.bass_utils import run_bass_kernel_spmd

# ---- problem constants (hardcoded per harness contract) ----
N = 50000
IN_D, HID1, HID2, OUT_D = 256, 128, 64, 32
NC_CORES = 8
NSH = N // NC_CORES  # 6250 dst nodes per core
NBLK = (NSH + 127) // 128  # 49 dst blocks per core
NPAIR = N // 2  # pair rows in the gather view (int16-safe: 25000 < 32767)
CHUNK_BLOCKS = 3  # dst blocks per gather chunk
FC = 64  # collapsed feature count
GT = 7  # tiles per gather sub-call: 57 descs/engine packet (<=64 HW cap)

BF16 = ml_dtypes.bfloat16

_tile_patched = False


def _patch_tile_drain():
    """walrus in this env rejects >~2 sem waits on one instruction; Tile's
    kernel-tail drain aggregates one wait per live semaphore. Move the excess
    onto dedicated single-wait SP nops that precede the drain."""
    global _tile_patched
    if _tile_patched:
        return
    _tile_patched = True
    _orig = tile.TileContext._drain_and_barrier

    def _patched(self, tick_clock, wait_clock):
        nc = self.nc
        nops = [nc.sync.nop(nofuse=True, hint=f"dw_{i}").ins for i in range(64)]
        _orig(self, tick_clock, wait_clock)
        ni = 0
        for inst in nc.cur_bb.bb.instructions:
            if "Drain" not in type(inst).__name__:
                continue
            ow = inst.sync_info.on_wait if inst.sync_info else []
            if len(ow) > 1:
                waits = list(ow)
                for w in waits[:-1]:
                    nops[ni].sync_info = mybir.SyncInfo(on_wait=[w], on_update=[])
                    ni += 1
                inst.sync_info.on_wait[:] = waits[-1:]

    tile.TileContext._drain_and_barrier = _patched


def _prep(x, edge_index, W1, b1, W2, b2, W_mu, b_mu, W_lv, b_lv):
    """Host-side graph partitioning + input staging. Returns (in_maps, plan)."""
    src = np.asarray(edge_index[0], dtype=np.int64)
    dst = np.asarray(edge_index[1], dtype=np.int64)

    # degrees include the self loop (handled on-device via identity matmul)
    deg = (np.bincount(dst, minlength=N) + 1).astype(np.float64)
    dinv = deg**-0.5
    invdeg = 1.0 / deg

    # sort real edges by (src-parity, dst): each (dst-block, parity) group
    # contiguous; parity selects gathered cols 0:64 vs 64:128 of a pair row
    par = src % 2
    key = par * N + dst
    order = np.argsort(key, kind="stable")
    s_sorted = src[order]
    d_sorted = dst[order]
    bnd = np.searchsorted(key[order], np.arange(2 * N + 1))

    # per-(core, block, parity) counts -> core-independent tile counts
    T = [[0, 0] for _ in range(NBLK)]
    counts = np.zeros((NC_CORES, NBLK, 2), dtype=np.int64)
    for c in range(NC_CORES):
        for b in range(NBLK):
            lo = c * NSH + b * 128
            hi = min(c * NSH + (b + 1) * 128, (c + 1) * NSH)
            for h in range(2):
                counts[c, b, h] = bnd[h * N + hi] - bnd[h * N + lo]
    MC = [[0, 0] for _ in range(NBLK)]
    for b in range(NBLK):
        for h in range(2):
            MC[b][h] = max(1, int(counts[:, b, h].max()))
            T[b][h] = -(-MC[b][h] // 128)

    TH = [sum(T[b][h] for b in range(NBLK)) for h in range(2)]
    toff = [[0] * NBLK, [0] * NBLK]
    for h in range(2):
        acc = 0
        for b in range(NBLK):
            toff[h][b] = acc
            acc += T[b][h]

    # per-core padded idx / dstloc streams (idx = pair row = src // 2)
    core_data = []
    for c in range(NC_CORES):
        idx_streams = []
        dl_streams = []
        for h in range(2):
            idx = np.zeros(TH[h] * 128, dtype=np.int16)
            dl = np.full(TH[h] * 128, -1.0, dtype=np.float32)
            for b in range(NBLK):
                lo = c * NSH + b * 128
                hi = min(c * NSH + (b + 1) * 128, (c + 1) * NSH)
                e0, e1 = bnd[h * N + lo], bnd[h * N + hi]
                cnt = e1 - e0
                off = toff[h][b] * 128
                idx[off : off + cnt] = (s_sorted[e0:e1] // 2).astype(np.int16)
                dl[off : off + cnt] = (d_sorted[e0:e1] - lo).astype(np.float32)
            packed = np.tile(np.ascontiguousarray(idx.reshape(-1, 16).T), (8, 1))
            idx_streams.append(packed)
            dl_streams.append(np.ascontiguousarray(dl.reshape(-1, 128).T).astype(BF16))
        core_data.append((idx_streams, dl_streams))

    # collapsed weights
    W1_ = np.asarray(W1, np.float64)
    W2_ = np.asarray(W2, np.float64)
    Wh = np.concatenate(
        [np.asarray(W_mu, np.float64), np.asarray(W_lv, np.float64)], axis=1
    )  # [64, 64]
    Wc = W1_ @ W2_ @ Wh  # [256, 64]
    wca = Wc[:128].astype(BF16)
    wcb = Wc[128:].astype(BF16)

    # host-side bias correction (zero for this module)
    r1 = (np.asarray(b1, np.float64) @ W2_) @ Wh  # [64]
    r0 = np.asarray(b2, np.float64) @ Wh + np.concatenate(
        [np.asarray(b_mu, np.float64), np.asarray(b_lv, np.float64)]
    )
    if np.any(r1) or np.any(r0):
        s_vec = dinv * (
            np.bincount(dst, weights=dinv[src], minlength=N) + dinv
        )
        bias_corr = (s_vec[:, None] * r1[None, :] + r0[None, :]).astype(np.float32)
    else:
        bias_corr = None

    iota_rep = np.tile(np.arange(128, dtype=np.float32), (128, 8)).astype(BF16)
    ident = np.eye(128, dtype=np.float32).astype(BF16)

    xf = np.asarray(x, np.float32)
    in_maps = []
    for c in range(NC_CORES):
        (idxA, idxB), (dlA, dlB) = core_data[c]
        own = slice(c * NSH, (c + 1) * NSH)
        xsh = np.zeros((IN_D, NBLK * 128), np.float32)
        xsh[:, :NSH] = xf[own].T
        tmp_iv = np.zeros(NBLK * 128, np.float64)
        tmp_dv = np.zeros(NBLK * 128, np.float64)
        tmp_iv[:NSH] = invdeg[own]
        tmp_dv[:NSH] = dinv[own]
        in_maps.append(
            {
                "xsh": xsh.astype(BF16),
                "iota": iota_rep,
                "ident": ident,
                "idxA": idxA,
                "idxB": idxB,
                "dlA": dlA,
                "dlB": dlB,
                "wca": wca,
                "wcb": wcb,
                "ivcol": np.ascontiguousarray(
                    tmp_iv.reshape(NBLK, 128).T
                ).astype(np.float32),
                "dvcol": np.ascontiguousarray(
                    tmp_dv.reshape(NBLK, 128).T
                ).astype(np.float32),
            }
        )

    plan = {"T": T, "TH": TH, "toff": toff, "MC": MC, "bias_corr": bias_corr}
    return in_maps, plan


def _build(plan):
    _patch_tile_drain()
    T, TH, toff, MC = plan["T"], plan["TH"], plan["toff"], plan["MC"]

    nc = bacc.Bacc("TRN2", num_swdge_queues=4, dynamic_dma_scratch_size=49152)
    f32, bf16, i16 = mybir.dt.float32, mybir.dt.bfloat16, mybir.dt.int16
    COPY = mybir.ActivationFunctionType.Copy

    xsh_e = nc.dram_tensor("xsh", [IN_D, NBLK * 128], bf16, kind="ExternalInput")
    iota_e = nc.dram_tensor("iota", [128, 1024], bf16, kind="ExternalInput")
    ident_e = nc.dram_tensor("ident", [128, 128], bf16, kind="ExternalInput")
    idxA_e = nc.dram_tensor("idxA", [128, TH[0] * 8], i16, kind="ExternalInput")
    idxB_e = nc.dram_tensor("idxB", [128, TH[1] * 8], i16, kind="ExternalInput")
    dlA_e = nc.dram_tensor("dlA", [128, TH[0]], bf16, kind="ExternalInput")
    dlB_e = nc.dram_tensor("dlB", [128, TH[1]], bf16, kind="ExternalInput")
    wca_e = nc.dram_tensor("wca", [128, FC], bf16, kind="ExternalInput")
    wcb_e = nc.dram_tensor("wcb", [128, FC], bf16, kind="ExternalInput")
    ivcol_e = nc.dram_tensor("ivcol", [128, NBLK], f32, kind="ExternalInput")
    dvcol_e = nc.dram_tensor("dvcol", [128, NBLK], f32, kind="ExternalInput")

    out_e = nc.dram_tensor("out", [NSH, FC], f32, kind="ExternalOutput")

    z0l_d = nc.dram_tensor("z0l_d", [NSH, FC], bf16)
    z0f_d = nc.dram_tensor("z0f_d", [NPAIR, 2 * FC], bf16, addr_space="Shared")
    z1l_d = nc.dram_tensor("z1l_d", [NSH, FC], bf16)
    z1f_d = nc.dram_tensor("z1f_d", [NPAIR, 2 * FC], bf16, addr_space="Shared")

    core_ids = list(range(NC_CORES))

    chunks = []
    b0 = 0
    while b0 < NBLK:
        chunks.append((b0, min(b0 + CHUNK_BLOCKS, NBLK)))
        b0 = min(b0 + CHUNK_BLOCKS, NBLK)

    with tile.TileContext(nc) as tc:
        with (
            tc.tile_pool(name="const", bufs=1) as pc,
            tc.tile_pool(name="xa", bufs=3) as px,
            tc.tile_pool(name="zl0", bufs=NBLK) as pzl0,
            tc.tile_pool(name="zl1", bufs=NBLK) as pzl1,
            tc.tile_pool(name="zb", bufs=4) as pz,
            tc.tile_pool(name="g", bufs=6) as pg,
            tc.tile_pool(name="sel", bufs=8) as psel,
            tc.tile_pool(name="psA", bufs=2, space="PSUM") as ppA,
            tc.tile_pool(name="psacc", bufs=6, space="PSUM") as ppa,
        ):
            nc.gpsimd.load_library(library_config.mlp)

            # ---- constants needed by phase A (loaded first so the HWDGE
            # queues prioritize the phase-A critical path)
            wca_t = pc.tile([128, FC], bf16)
            nc.sync.dma_start(out=wca_t[:], in_=wca_e[:])
            wcb_t = pc.tile([128, FC], bf16)
            nc.sync.dma_start(out=wcb_t[:], in_=wcb_e[:])
            dvcol_t = pc.tile([128, NBLK], f32)
            nc.sync.dma_start(out=dvcol_t[:], in_=dvcol_e[:])

            z0loc = [None] * NBLK
            z1loc = [None] * NBLK

            # ---- phase A: z0 shard = dinv * (x_shard @ Wc)
            with nc.named_scope("phaseA"):
                done = 0
                while done < NBLK:
                    nb_cnt = min(8, NBLK - done)
                    c0 = done * 128
                    cols = nb_cnt * 128
                    xa = px.tile([128, 1024], bf16, tag="xa")
                    xb = px.tile([128, 1024], bf16, tag="xb")
                    nc.sync.dma_start(
                        out=xa[:, :cols], in_=xsh_e[0:128, c0 : c0 + cols]
                    )
                    nc.scalar.dma_start(
                        out=xb[:, :cols], in_=xsh_e[128:256, c0 : c0 + cols]
                    )
                    for j in range(nb_cnt):
                        gb = done + j
                        rows = min(128, NSH - gb * 128)
                        zp = ppA.tile([128, FC], f32, space="PSUM", tag="zp")
                        nc.tensor.matmul(
                            out=zp[:],
                            lhsT=xa[:, j * 128 : (j + 1) * 128],
                            rhs=wca_t[:],
                            start=True,
                            stop=False,
                        )
                        nc.tensor.matmul(
                            out=zp[:],
                            lhsT=xb[:, j * 128 : (j + 1) * 128],
                            rhs=wcb_t[:],
                            start=False,
                            stop=True,
                        )
                        z0s = pzl0.tile([128, FC], bf16, tag="z0s")
                        z0loc[gb] = z0s
                        nc.scalar.activation(
                            z0s[:], zp[:], COPY,
                            scale=dvcol_t[:, gb : gb + 1],
                        )
                        nc.sync.dma_start(
                            out=z0l_d[gb * 128 : gb * 128 + rows], in_=z0s[:rows]
                        )
                    done += nb_cnt

            with nc.named_scope("ag0"):
                nc.gpsimd.collective_compute(
                    "AllGather",
                    mybir.AluOpType.bypass,
                    ins=[z0l_d[:]],
                    outs=[z0f_d[:]],
                    replica_groups=[core_ids],
                )

            # ---- spmm-only constants, emitted after phase A so their DMAs
            # queue behind the phase-A loads/stores
            iota_t = pc.tile([128, 1024], bf16)
            nc.sync.dma_start(out=iota_t[:], in_=iota_e[:])
            ident_t = pc.tile([128, 128], bf16)
            nc.sync.dma_start(out=ident_t[:], in_=ident_e[:])
            idxA_t = pc.tile([128, TH[0] * 8], i16)
            nc.sync.dma_start(out=idxA_t[:], in_=idxA_e[:])
            idxB_t = pc.tile([128, TH[1] * 8], i16)
            nc.sync.dma_start(out=idxB_t[:], in_=idxB_e[:])
            dlA_t = pc.tile([128, TH[0]], bf16)
            nc.sync.dma_start(out=dlA_t[:], in_=dlA_e[:])
            dlB_t = pc.tile([128, TH[1]], bf16)
            nc.sync.dma_start(out=dlB_t[:], in_=dlB_e[:])
            ivcol_t = pc.tile([128, NBLK], f32)
            nc.sync.dma_start(out=ivcol_t[:], in_=ivcol_e[:])

            # pre-zero the gather pool's buffer rotation (overlaps phase A /
            # AG0) so un-gathered trimmed pad slots can't feed NaNs to the PE
            ctA_max = max(
                sum(T[b][0] for b in range(cb0, cb1)) for (cb0, cb1) in chunks
            )
            ctB_max = max(
                sum(T[b][1] for b in range(cb0, cb1)) for (cb0, cb1) in chunks
            )
            for _ in range(6):
                gz = pg.tile([128, ctA_max, 128], bf16, tag="gA")
                nc.vector.memset(gz[:], 0.0)
                gz = pg.tile([128, ctB_max, 128], bf16, tag="gB")
                nc.vector.memset(gz[:], 0.0)

            # ---- shared SpMM: acc[dst_block] = z_self[block] + sum_e msg[e]
            def spmm(src_d, zloc, scale_t, store, qoff=0):
                qn = qoff
                for ci, (cb0, cb1) in enumerate(chunks):
                    ctA = sum(T[b][0] for b in range(cb0, cb1))
                    ctB = sum(T[b][1] for b in range(cb0, cb1))
                    offA, offB = toff[0][cb0], toff[1][cb0]
                    gA = pg.tile([128, ctA, 128], bf16, tag="gA")
                    gB = pg.tile([128, ctB, 128], bf16, tag="gB")
                    for h, g, idx_t, off in (
                        (0, gA, idxA_t, offA),
                        (1, gB, idxB_t, offB),
                    ):
                        for b in range(cb0, cb1):
                            tb0 = toff[h][b] - off
                            nt = T[b][h]
                            mc = MC[b][h]
                            t0 = 0
                            while t0 < nt:
                                tn = min(GT, nt - t0)
                                ni = max(1, min(tn * 128, mc - t0 * 128))
                                nc.gpsimd.dma_gather(
                                    g[:, tb0 + t0 : tb0 + t0 + tn, :], src_d,
                                    idx_t[
                                        :,
                                        (toff[h][b] + t0) * 8 :
                                        (toff[h][b] + t0 + tn) * 8,
                                    ],
                                    ni, ni, 128,
                                    single_packet=True, queue_num=qn % 4,
                                )
                                qn += 1
                                t0 += tn
                    for b in range(cb0, cb1):
                        acc = ppa.tile([128, FC], f32, space="PSUM", tag="acc")
                        n_mm = 1 + T[b][0] + T[b][1]
                        nc.tensor.matmul(
                            out=acc[:],
                            lhsT=ident_t[:],
                            rhs=zloc[b][:],
                            start=True,
                            stop=False,
                        )
                        mi = 1
                        for h, g, dl_t, off0 in (
                            (0, gA, dlA_t, offA),
                            (1, gB, dlB_t, offB),
                        ):
                            tloc0 = toff[h][b] - off0
                            nt = T[b][h]
                            done = 0
                            while done < nt:
                                k = min(8, nt - done)
                                sel = psel.tile([128, k, 128], bf16, tag="sel")
                                nc.vector.tensor_tensor(
                                    out=sel[:],
                                    in0=dl_t[
                                        :, toff[h][b] + done : toff[h][b] + done + k
                                    ].to_broadcast([128, k, 128]),
                                    in1=iota_t[:, : k * 128],
                                    op=mybir.AluOpType.is_equal,
                                )
                                for q in range(k):
                                    nc.tensor.matmul(
                                        out=acc[:],
                                        lhsT=sel[:, q, :],
                                        rhs=g[
                                            :, tloc0 + done + q,
                                            h * FC : h * FC + FC,
                                        ],
                                        start=False,
                                        stop=(mi == n_mm - 1),
                                    )
                                    mi += 1
                                done += k
                        store(b, acc, scale_t)

            # ---- SpMM1 -> z1 table shard; AllGather
            def store_z1(b, acc, scale_t):
                rows = min(128, NSH - b * 128)
                z1s = pzl1.tile([128, FC], bf16, tag="z1s")
                z1loc[b] = z1s
                nc.scalar.activation(
                    z1s[:], acc[:], COPY, scale=scale_t[:, b : b + 1]
                )
                nc.sync.dma_start(
                    out=z1l_d[b * 128 : b * 128 + rows], in_=z1s[:rows]
                )

            with nc.named_scope("spmm1"):
                spmm(z0f_d[:], z0loc, ivcol_t, store_z1, qoff=0)

            with nc.named_scope("ag1"):
                nc.gpsimd.collective_compute(
                    "AllGather",
                    mybir.AluOpType.bypass,
                    ins=[z1l_d[:]],
                    outs=[z1f_d[:]],
                    replica_groups=[core_ids],
                )

            # ---- SpMM2 -> final output
            def store_out(b, acc, scale_t):
                rows = min(128, NSH - b * 128)
                os_ = pz.tile([128, FC], f32, tag="outs")
                nc.scalar.activation(
                    os_[:], acc[:], COPY, scale=scale_t[:, b : b + 1]
                )
                nc.sync.dma_start(
                    out=out_e[b * 128 : b * 128 + rows], in_=os_[:rows]
                )

            with nc.named_scope("spmm2"):
                spmm(z1f_d[:], z1loc, dvcol_t, store_out, qoff=2)

    nc.compile()
    return nc


_CACHE = {}


def kernel(**inputs):
    in_maps, plan = _prep(**inputs)
    key = tuple(tuple(t) for t in plan["MC"])
    if key not in _CACHE:
        _CACHE[key] = _build(plan)
    nc = _CACHE[key]
    res = run_bass_kernel_spmd(nc, in_maps, list(range(NC_CORES)))
    out = np.concatenate(
        [res.results[c]["out"] for c in range(NC_CORES)], axis=0
    )
    if plan["bias_corr"] is not None:
        out = out + plan["bias_corr"]
    mu = np.ascontiguousarray(out[:, :OUT_D])
    lv = np.ascontiguousarray(out[:, OUT_D:])
    return (mu, lv)
